# revision 1
# baseline (speedup 1.0000x reference)
"""Trainium2 Bass kernel for nn_NeuralMemory (B=4, N=1024, D=128, DEPTH=4).

Sharding: 8 cores, core c handles batch b = c//2. The store phase
(per-token grads of the 4 memory weights, summed over the sequence) is
computed redundantly by both cores of a pair -- the grad sum is
permutation invariant over tokens, so each core is fed its batch's
sequence with its own retrieval half rotated to the front and retrieves
tokens [0:512) of its view. No cross-core communication (a pair-wise
AllReduce has a ~10us floor, worse than the duplicated compute).

Layout: activations are feature-major [D=128 partitions, tokens]; the
store phase runs in two 512-token tiles. Layer matmuls are
matmul(out^T, lhsT=W, rhs=X^T) with float32r operands (~2e-4
per-matmul rel err on HW). dW_i = A_i^T @ G_i contracts over tokens,
so A/G get bf16 copies rotated token-major via PE transposes (4 chunks
per PSUM bank); dW matmuls run bf16 with fp32 PSUM accumulation.
dW3/dW2/dW1 and M = S^T @ G0 share one PSUM bank (a single
accumulation group). Tile-1's H tiles borrow the dW-transpose ("tr")
PSUM banks, which idle until mid-kernel, so both tiles' forwards
pipeline; a few dummy matmuls at t=0 hold the PE HAM clock window busy
so the first transposes run at full clock.

K is never materialized: H0 = S @ (Wk @ w0) with the [D,D] composition
on-chip, and the retrieval's first layer is rewritten
  X1 = X0 @ w0 + (X0 @ Wk^T) @ M,   X0 = S @ wq
so X0 and P^T = Wk @ X0^T are computed early and only the tiny
M-eviction sits on the critical tail (U0/dW0 never materialize).
V is folded into H3's PSUM accumulation with a negated Wv (G3 raw =
H3 - V straight out of one bank; the 2/D scale lives in w3^T and in
the a3 bf16 cast).

All weights arrive in ONE packed DRAM tensor (HWDGE dispatch is ~625ns
per dma_start, serialized); seq arrives in 2 halves plus a casting
SWDGE bf16 copy.

ACT-table discipline: all forward Silus before any Derivative_silu
(H0..H2 evicted to SBUF), and a dummy Silu reloads the silu table
during the dW phase so the retrieval tail pays no table load.
"""

import numpy as np

import concourse.bass as bass
import concourse.mybir as mybir
import concourse.tile as tile
from concourse import bacc
from concourse.bass import ts
from concourse.bass_utils import run_bass_kernel_spmd
from concourse.masks import make_identity

B, N, D = 4, 1024, 128
DEPTH = 4
NCORES = 8
NT = 512            # tokens retrieved per core (half a batch)
TT = 512            # store-phase token tile
NTI = N // TT       # store tiles
NCHUNK = N // 128   # 8 token chunks of 128
RH = 256            # retrieval sub-tile
WPACK = 4 * D + D + 2 * D   # w0..w3 | wq | wkv

f32 = mybir.dt.float32
f32r = mybir.dt.float32r
bf16 = mybir.dt.bfloat16

AF = mybir.ActivationFunctionType
ALU = mybir.AluOpType

TM_DT = bf16


def _build_program(reps=1):
    nc = bacc.Bacc(
        "TRN2",
        target_bir_lowering=False,
        debug=False,
        enable_asserts=False,
        num_devices=NCORES,
    )

    seq = nc.dram_tensor("seq", [N, D], f32, kind="ExternalInput").ap()
    wp_dr = nc.dram_tensor("wpack", [D, WPACK], f32, kind="ExternalInput").ap()
    out_dr = nc.dram_tensor("out", [NT, D], f32, kind="ExternalOutput").ap()

    with tile.TileContext(nc) as tc:
        for _ in range(reps):
            _emit(tc, seq, wp_dr, out_dr)

    nc.compile()
    return nc


def _emit(tc, seq, wp_dr, out_dr):
    nc = tc.nc
    from contextlib import ExitStack

    from concourse.tile_rust import add_dep_helper as _dep  # type: ignore

    with ExitStack() as ctx:
        consts = ctx.enter_context(tc.tile_pool(name="consts", bufs=1))
        big = ctx.enter_context(tc.tile_pool(name="big", bufs=1))
        # PSUM banks: mm(2) + hold(2) + tr(3) + dw(1) = 8
        pp = ctx.enter_context(tc.tile_pool(name="pp", bufs=1, space="PSUM"))

        def pmm(name, w=512):
            return pp.tile([128, w], f32, tag="mm", bufs=2, name=name)

        def phold(name, w=512):
            return pp.tile([128, w], f32, tag="hold", bufs=2, name=name)

        def ptr(name):
            return pp.tile([128, 512], TM_DT, tag="tr", bufs=3, name=name)

        # tiny scratch silu pulls the first ACT table load off the
        # critical path (runs during the DMAs)
        scr = consts.tile([128, 1], f32, tag="scr")
        scr2 = consts.tile([128, 1], f32, tag="scr2")
        nc.gpsimd.memset(scr[:], 0.0)
        nc.scalar.activation(scr2[:], scr[:], AF.Silu)

        # PE warm-up: keep the HAM clock window busy before real work so
        # the S^T transposes and first matmuls run at full clock
        wupa = consts.tile([128, 128], f32r, tag="wupa")
        nc.gpsimd.memset(wupa[:].bitcast(f32), 0.0)
        wupp = pp.tile([128, 512], f32, tag="tr", bufs=3, name="wupp")
        for k in range(3):
            nc.tensor.matmul(
                wupp[:, 0:128], wupa[:], wupa[:],
                skip_group_check=True,
            )

        ident = consts.tile([128, 128], f32, tag="ident")
        make_identity(nc, ident)
        ident_b = consts.tile([128, 128], bf16, tag="ident_b")
        nc.gpsimd.tensor_copy(ident_b[:], ident[:])

        # ---- DMAs ordered by need ----
        wp = consts.tile([D, WPACK], f32, tag="wp")
        nc.sync.dma_start(wp[:], wp_dr)
        w_sb = [wp[:, ts(i, D)] for i in range(4)]
        wq_sb = wp[:, ts(4, D)]
        wkv_sb = wp[:, 5 * D : 7 * D]

        s_tm = big.tile([128, NCHUNK, 128], f32, tag="s_tm")
        seq_r = seq.rearrange("(c p) d -> p c d", p=128)
        nc.sync.dma_start(s_tm[:, 0:4], seq_r[:, 0:4])
        nc.sync.dma_start(s_tm[:, 4:8], seq_r[:, 4:8])
        s_tmb = big.tile([128, NCHUNK, 128], bf16, tag="s_tmb")

        # persistent SBUF activations (feature-major)
        st = big.tile([128, N], f32r, tag="st")
        a1 = big.tile([128, N], f32r, tag="a1")
        a2 = big.tile([128, N], f32r, tag="a2")
        a3 = big.tile([128, N], f32r, tag="a3")
        hsb = big.tile([128, 3, N], f32, tag="hsb")     # H0..H2 in SBUF
        sp0 = big.tile([128, N], f32, tag="sp0")
        sp1 = big.tile([128, N], f32, tag="sp1")
        sp2 = big.tile([128, N], f32, tag="sp2")
        g1 = big.tile([128, N], f32r, tag="g1")
        g2 = big.tile([128, N], f32r, tag="g2")
        g3 = big.tile([128, N], f32r, tag="g3")         # raw H3 - V
        # bf16 copies for the dW path (a3b carries the 2/D scale)
        a1b = big.tile([128, N], TM_DT, tag="a1b")
        a2b = big.tile([128, N], TM_DT, tag="a2b")
        a3b = big.tile([128, N], TM_DT, tag="a3b")
        g0b = big.tile([128, N], TM_DT, tag="g0b")
        g1b = big.tile([128, N], TM_DT, tag="g1b")
        g2b = big.tile([128, N], TM_DT, tag="g2b")
        g3b = big.tile([128, N], TM_DT, tag="g3b")

        wt = big.tile([128, 3, 128], f32r, tag="wt")    # w1^T,w2^T,w3^T*(2/D)
        wk_t = big.tile([128, 128], f32, tag="wk_t")    # Wk^T (fp32)
        wk_tr = big.tile([128, 128], f32r, tag="wk_tr")  # Wk^T (f32r)
        w0eff = big.tile([128, 128], f32r, tag="w0eff")  # Wk @ w0
        w0r = big.tile([128, 128], f32r, tag="w0r")
        wqr = big.tile([128, 128], f32r, tag="wqr")
        wv_r = big.tile([D, D], f32r, tag="wv_r")       # -Wv
        w_r = [None] + [
            big.tile([D, D], f32r, name=f"wr{i}", tag=f"wr{i}") for i in (1, 2, 3)
        ]
        for i in (1, 2, 3):
            nc.vector.tensor_copy(w_r[i][:], w_sb[i])
        # negated so V accumulates as -V into H3's PSUM bank
        nc.vector.tensor_scalar_mul(wv_r[:], wkv_sb[:, D : 2 * D], -1.0)
        nc.vector.tensor_copy(w0r[:], w_sb[0])
        nc.vector.tensor_copy(wqr[:], wq_sb)

        silu_insts = []
        dsilu_insts = []

        # ---- S^T (before weight-gated work: pool slots stay free) (fp32 PE transposes, evictions round to f32r) ----
        for g in range(NCHUNK // 4):
            p = pmm(f"p_st{g}")
            for j in range(4):
                nc.tensor.transpose(p[:, ts(j, 128)], s_tm[:, g * 4 + j], ident)
            nc.vector.tensor_copy(st[:, ts(g, 512)], p[:])

        # ---- setup transposes + W0eff ----
        p = pmm("p_tr1")
        nc.tensor.transpose(p[:, ts(0, 128)], wkv_sb[:, 0:D], ident)
        for i in range(2):
            nc.tensor.transpose(p[:, ts(1 + i, 128)], w_sb[1 + i], ident)
        nc.tensor.transpose(p[:, ts(3, 128)], w_sb[3], ident)
        nc.vector.tensor_copy(wk_t[:], p[:, 0:128])
        nc.vector.tensor_copy(wk_tr[:], p[:, 0:128])
        nc.vector.tensor_copy(
            wt[:, 0:2], p[:, 128:384].rearrange("p (c d) -> p c d", d=128)
        )
        nc.scalar.activation(wt[:, 2], p[:, 384:512], AF.Copy, scale=2.0 / D)

        p = pmm("p_w0eff")
        nc.tensor.matmul(p[:, 0:128], wk_t[:], w_sb[0])
        nc.vector.tensor_copy(w0eff[:], p[:, 0:128])

        # ---- X0^T = wq^T S^T and P^T = Wk X0^T (ACT evictions: DVE is the
        # fwd-setup bottleneck and ACT idles until the first Silu) ----
        x0 = big.tile([128, NT], f32r, tag="x0")
        px = pmm("p_x0")
        nc.tensor.matmul(px[:], wqr[:], st[:, 0:NT])
        nc.vector.tensor_copy(x0[:], px[:])
        pt = big.tile([128, NT], f32r, tag="pt")
        px = pmm("p_pt")
        nc.tensor.matmul(px[:], wk_tr[:], x0[:])
        nc.vector.tensor_copy(pt[:], px[:])

        # ---- forward: all Silus first; H2 held in PSUM, H0/H1 to SBUF ----
        holds = {}
        for t in range(NTI):
            sl = ts(t, TT)
            hloc = []
            for li in range(3):
                wst = (w0eff, w_r[1], w_r[2])[li]
                rhs = (st, a1, a2)[li]
                if li == 2:
                    h = phold(f"h{li}_{t}", TT)
                elif t == 1:
                    # tile-1 H0/H1 borrow the (idle until dW) tr banks
                    h = pp.tile([128, TT], f32, tag="tr", bufs=3, name=f"h{li}_{t}")
                else:
                    h = pmm(f"h{li}_{t}", TT)
                nc.tensor.matmul(h[:], wst[:], rhs[:, sl])
                dst = (a1, a2, a3)[li]
                silu_insts.append(nc.scalar.activation(dst[:, sl], h[:], AF.Silu))
                if li == 2:
                    hloc.append(h[:])
                else:
                    nc.vector.tensor_copy(hsb[:, li, sl], h[:])
                    hloc.append(hsb[:, li, sl])
            # H3 - V accumulated in one PSUM bank (wv_r is negated)
            if t == 1:
                h3 = pp.tile([128, TT], f32, tag="tr", bufs=3, name=f"h3_{t}")
            else:
                h3 = pmm(f"h3_{t}", TT)
            nc.tensor.matmul(h3[:], w_r[3][:], a3[:, sl], start=True, stop=False)
            nc.tensor.matmul(h3[:], wv_r[:], st[:, sl], start=False, stop=True)
            nc.vector.tensor_copy(g3[:, sl], h3[:])     # raw H3 - V
            nc.gpsimd.tensor_copy(a1b[:, sl], a1[:, sl].bitcast(f32))
            nc.gpsimd.tensor_copy(a2b[:, sl], a2[:, sl].bitcast(f32))
            nc.gpsimd.tensor_scalar_mul(
                a3b[:, sl], a3[:, sl].bitcast(f32), 2.0 / D
            )
            nc.gpsimd.tensor_copy(g3b[:, sl], g3[:, sl].bitcast(f32))
            holds[t] = hloc

        # ---- backward: Derivative_silu after all Silus + chains ----
        for t in range(NTI):
            di = nc.scalar.activation(
                sp2[:, ts(t, TT)], holds[t][2], AF.Derivative_silu
            )
            dsilu_insts.append(di)
        dsilu_insts.append(
            nc.scalar.activation(sp1[:], hsb[:, 1, :], AF.Derivative_silu)
        )
        dsilu_insts.append(
            nc.scalar.activation(sp0[:], hsb[:, 0, :], AF.Derivative_silu)
        )
        for t in range(NTI):
            sl = ts(t, TT)

            c2 = pmm(f"c2_{t}", TT)
            nc.tensor.matmul(c2[:], wt[:, 2], g3[:, sl])
            nc.vector.tensor_mul(g2[:, sl], c2[:], sp2[:, sl])

            c1 = pmm(f"c1_{t}", TT)
            nc.tensor.matmul(c1[:], wt[:, 1], g2[:, sl])
            nc.vector.tensor_mul(g1[:, sl], c1[:], sp1[:, sl])

            c0 = pmm(f"c0_{t}", TT)
            nc.tensor.matmul(c0[:], wt[:, 0], g1[:, sl])
            nc.vector.tensor_mul(g0b[:, sl], c0[:], sp0[:, sl])  # bf16 direct
            nc.gpsimd.tensor_copy(g2b[:, sl], g2[:, sl].bitcast(f32))
            nc.gpsimd.tensor_copy(g1b[:, sl], g1[:, sl].bitcast(f32))

        for di in dsilu_insts:
            _dep(di.ins, silu_insts[-1].ins, sync=False, reason="act-table order")

        # bf16 seq copy for the M matmuls -- held back (dep on the first
        # Silu) so its transfer doesn't delay the seq/weight DMAs at startup
        _stmb_dma = nc.gpsimd.dma_start(s_tmb[:], seq_r)
        _dep(_stmb_dma.ins, silu_insts[0].ins, sync=False,
             reason="defer bf16 seq copy off the startup DMA path")

        # ---- token-major transposes + dW accumulation ---------------------
        a_tm = [None] + [
            big.tile([128, N], TM_DT, name=f"atm{i}", tag=f"atm{i}") for i in (1, 2, 3)
        ]
        g_tm = [
            big.tile([128, N], TM_DT, name=f"gtm{i}", tag=f"gtm{i}") for i in range(4)
        ]
        u = [
            None,
            consts.tile([D, D], f32r, name="u1", tag="u1"),
            consts.tile([D, D], f32r, name="u2", tag="u2"),
            consts.tile([D, D], f32r, name="u3", tag="u3"),
        ]

        # reload the silu table during the dW phase, off the tail
        scr3 = consts.tile([128, 1], f32, tag="scr3")
        dummy = nc.scalar.activation(scr3[:], scr[:], AF.Silu)
        _dep(dummy.ins, dsilu_insts[-1].ins, sync=False, reason="act-table order")

        evict_flip = [0]

        def transpose_half(src, dst, h, name):
            p = ptr(name)
            for j in range(4):
                c = h * 4 + j
                nc.tensor.matmul(
                    p[:, ts(j, 128)], src[:, ts(c, 128)], ident_b[:],
                    is_transpose=True,
                )
            if evict_flip[0] % 3 == 2:
                nc.scalar.activation(dst[:, ts(h, 512)], p[:], AF.Copy)
            else:
                nc.vector.tensor_copy(dst[:, ts(h, 512)], p[:])
            evict_flip[0] += 1

        # dW3/dW2/dW1 and M share one PSUM bank (one accumulation group)
        acc = pp.tile([128, 4, 128], f32, tag="dw", bufs=1, name="dwacc")
        first = [True]

        for i, (ab, gb, atm, gtm, slot) in enumerate(
            (
                (a3b, g3b, a_tm[3], g_tm[3], 0),
                (a2b, g2b, a_tm[2], g_tm[2], 1),
                (a1b, g1b, a_tm[1], g_tm[1], 2),
            )
        ):
            for h in range(2):
                transpose_half(ab, atm, h, f"p_a{i}{h}")
                transpose_half(gb, gtm, h, f"p_g{i}{h}")
                for j in range(4):
                    c = h * 4 + j
                    nc.tensor.matmul(
                        acc[:, slot],
                        atm[:, ts(c, 128)],
                        gtm[:, ts(c, 128)],
                        start=first[0],
                        stop=False,
                    )
                    first[0] = False

        # M = S^T @ G0 into acc slot 3 (last writes of the bank group)
        for h in range(2):
            transpose_half(g0b, g_tm[0], h, f"p_g0{h}")
            for j in range(4):
                c = h * 4 + j
                nc.tensor.matmul(
                    acc[:, 3],
                    s_tmb[:, c],
                    g_tm[0][:, ts(c, 128)],
                    start=False,
                    stop=(h == 1 and j == 3),
                )
        m_r = big.tile([128, 128], f32r, tag="m_r")
        nc.vector.tensor_copy(m_r[:], acc[:, 3])
        for slot, i in ((2, 1), (1, 2), (0, 3)):
            nc.vector.tensor_add(u[i][:], acc[:, slot], w_sb[i])

        # ---- retrieval: X1 = X0 @ w0 + P @ M, then layers 2..4 ------------
        r1 = big.tile([128, NT], f32r, tag="r1")
        r2 = big.tile([128, NT], f32r, tag="r2")
        r3 = big.tile([128, NT], f32r, tag="r3")
        o_tm = big.tile([128, NT // 128, 128], f32, tag="o_tm")
        out_r = out_dr.rearrange("(c p) d -> p c d", p=128)

        nh = NT // RH
        px1s = []
        for hh in range(nh):
            sl = ts(hh, RH)
            px = phold(f"px1_{hh}", RH)
            # term 1 (X0 @ w0) has no M dependency -- runs during the dW phase
            nc.tensor.matmul(px[:], w0r[:], x0[:, sl], start=True, stop=False)
            px1s.append(px)
        for hh in range(nh):
            sl = ts(hh, RH)
            px = px1s[hh]
            nc.tensor.matmul(px[:], m_r[:], pt[:, sl], start=False, stop=True)
            nc.scalar.activation(r1[:, sl], px[:], AF.Silu)
        for hh in range(nh):
            sl = ts(hh, RH)
            px = phold(f"px2_{hh}", RH)
            nc.tensor.matmul(px[:], u[1][:], r1[:, sl])
            nc.scalar.activation(r2[:, sl], px[:], AF.Silu)
        for hh in range(nh):
            sl = ts(hh, RH)
            px = pmm(f"px3_{hh}", RH)
            nc.tensor.matmul(px[:], u[2][:], r2[:, sl])
            nc.scalar.activation(r3[:, sl], px[:], AF.Silu)
        for hh in range(nh):
            po = pmm(f"po{hh}", RH)
            for j in range(RH // 128):
                c = hh * (RH // 128) + j
                nc.tensor.matmul(
                    po[:, ts(j, 128)],
                    r3[:, ts(c, 128)],
                    u[3][:],
                    start=(j == 0),
                    stop=(j == RH // 128 - 1),
                )
            nc.vector.tensor_copy(
                o_tm[:, 2 * hh : 2 * hh + 2],
                po[:].rearrange("p (c d) -> p c d", d=128),
            )
            nc.sync.dma_start(
                out_r[:, 2 * hh : 2 * hh + 2], o_tm[:, 2 * hh : 2 * hh + 2]
            )


_CACHE = {}


def _get_nc():
    if "nc" not in _CACHE:
        _CACHE["nc"] = _build_program()
    return _CACHE["nc"]


def kernel(seq, w0, w1, w2, w3, wq, wkv):
    nc = _get_nc()
    seq = np.ascontiguousarray(np.asarray(seq, np.float32))
    wpack = np.ascontiguousarray(
        np.concatenate(
            [np.asarray(x, np.float32) for x in (w0, w1, w2, w3, wq, wkv)], axis=1
        )
    )

    in_maps = []
    for c in range(NCORES):
        b, h = c // 2, c % 2
        if h == 0:
            s = seq[b]
        else:
            # rotate: retrieval half first; grad sum is order-invariant
            s = np.concatenate([seq[b, NT:], seq[b, :NT]], axis=0)
        in_maps.append({"seq": np.ascontiguousarray(s), "wpack": wpack})

    res = run_bass_kernel_spmd(nc, in_maps, core_ids=list(range(NCORES)))
    _CACHE["last_results"] = res

    out = np.empty((B, N, D), np.float32)
    for c in range(NCORES):
        b, h = c // 2, c % 2
        out[b, h * NT : (h + 1) * NT] = res.results[c]["out"]
    return out



# revision 46
# speedup vs baseline: 1.2001x; 1.2001x over previous
"""Trainium2 Bass kernel for nn_NeuralMemory (B=4, N=1024, D=128, DEPTH=4).

Sharding: 8 cores, core c handles batch b = c//2; the store phase is
computed redundantly by both cores of a pair (grad sum is token-order
invariant; core h=1 sees its batch rotated so its retrieval half is
tokens [0:512) of its view). No cross-core communication.

v2 design vs the f32r baseline:
- bf16 store phase: Silu writes bf16 activations directly (no f32r
  copies, no gpsimd casts); fwd/bwd chain matmuls run bf16.
- 2/D is folded into the H3-layer weights (w3s = 2/D*w3, wv_s =
  -2/D*Wv), so the H3 PSUM accumulation yields g3 = 2/D*(H3-V) directly.
- dsilu inputs are recomputed on the PE (h1' = a1@w1, h0 token-major)
  instead of persisting H0/H1 -> no fwd PSUM evictions for them.
- The last backward step runs token-major: c0_tm = g1b-chunks @ w1^T,
  sp0_tm = dsilu(h0_tm), g0_tm = c0_tm*sp0_tm feeds M = S^T@G0 without
  a transpose on the critical path.
- Inputs arrive via four parallel DMA queues (SWDGE: [wkv|w0] + seq
  half 1 converting f32->bf16; HWDGE SP: seq half 0; HWDGE ACT: w1;
  HWDGE DVE: [w2|w3|wq]).
- Retrieval keeps f32r (u_i = w_i + dW_i in f32r) except the final
  layer (r3/u3 bf16 so the 128-wide output matmuls run at 1 cyc/row).
- ACT-table epochs: all Silus, then all Derivative_silus (table load
  hidden under c2/transposes), then a dummy Silu reload during the dW
  phase so retrieval Silus pay no load.
"""

import numpy as np

import concourse.bass as bass
import concourse.mybir as mybir
import concourse.tile as tile
from concourse import bacc
from concourse.bass import ts
from concourse.bass_utils import run_bass_kernel_spmd
from concourse.masks import make_identity

B, N, D = 4, 1024, 128
NT = 512            # tokens retrieved per core (half a batch)
TT = 512            # store-phase token tile
NTI = N // TT       # store tiles
NCHUNK = N // 128   # token chunks of 128
RH = 256            # retrieval sub-tile
NCORES = 8
SC = 2.0 / D

# wpack column layout (built host-side): [wkv | w0 | w1 | w2 | w3 | wq]
C_WKV, C_W0, C_W1, C_W2, C_W3, C_WQ = 0, 256, 384, 512, 640, 768
WPACK = 896

f32 = mybir.dt.float32
f32r = mybir.dt.float32r
bf16 = mybir.dt.bfloat16

AF = mybir.ActivationFunctionType
ALU = mybir.AluOpType


def _build_program(reps=1):
    nc = bacc.Bacc(
        "TRN2",
        target_bir_lowering=False,
        debug=False,
        enable_asserts=False,
        num_devices=NCORES,
    )

    seq = nc.dram_tensor("seq", [N, D], f32, kind="ExternalInput").ap()
    wp_dr = nc.dram_tensor("wpack", [D, WPACK], f32, kind="ExternalInput").ap()
    out_dr = nc.dram_tensor("out", [NT, D], f32, kind="ExternalOutput").ap()

    with tile.TileContext(nc) as tc:
        for _ in range(reps):
            _emit(tc, seq, wp_dr, out_dr)

    nc.compile()
    return nc


def _emit(tc, seq, wp_dr, out_dr):
    nc = tc.nc
    from contextlib import ExitStack

    from concourse.tile_rust import add_dep_helper as _dep  # type: ignore

    with ExitStack() as ctx:
        consts = ctx.enter_context(tc.tile_pool(name="consts", bufs=1))
        big = ctx.enter_context(tc.tile_pool(name="big", bufs=1))
        # PSUM banks: mm(2) + hold(2) + vh(2) + dw(1) + wtr(1) = 8
        pp = ctx.enter_context(tc.tile_pool(name="pp", bufs=1, space="PSUM"))

        def pmm(name, w=512, dt=f32):
            return pp.tile([128, w], dt, tag="mm", bufs=2, name=name)

        def phold(name, w=512, dt=f32):
            return pp.tile([128, w], dt, tag="hold", bufs=2, name=name)

        def pvh(name, w=512, dt=f32):
            return pp.tile([128, w], dt, tag="vh", bufs=2, name=name)

        def pdw(name, w=512, dt=bf16):
            return pp.tile([128, w], dt, tag="dw", bufs=1, name=name)

        # PE warm-up ASAP: sets pe_busy_start early so real work runs at
        # full clock (>3us ramp)
        wupa = consts.tile([128, 128], f32r, tag="wupa")
        nc.gpsimd.memset(wupa[:].bitcast(f32), 0.0)
        wupp = pp.tile([128, 512], f32, tag="wtr", bufs=1, name="wupp")
        for k in range(3):
            nc.tensor.matmul(
                wupp[:, 0:128], wupa[:], wupa[:],
                skip_group_check=True,
            )

        # tiny scratch silu pulls the first ACT table load off the
        # critical path (runs during the DMAs)
        scr = consts.tile([128, 1], f32, tag="scr")
        scr2 = consts.tile([128, 1], f32, tag="scr2")
        nc.gpsimd.memset(scr[:], 0.0)
        first_silu_load = nc.scalar.activation(scr2[:], scr[:], AF.Silu)

        # ---- input DMAs across queues (emitted before identity setup so
        # the Pool sequencer reaches the SWDGE desc-gens early) ----------
        wp = consts.tile([D, WPACK], f32, tag="wp")
        s_tmb = big.tile([128, NCHUNK, 128], bf16, tag="s_tmb")  # token-major S
        seq_r = seq.rearrange("(c p) d -> p c d", p=128)

        # HWDGE SP: [wkv|w0] f32 (first on the DMA device -> w0eff chain);
        # SWDGE (gpsimd) converts seq f32->bf16, halves in tile order;
        # HWDGE ACT: remaining weights.
        nc.sync.dma_start(wp[:, C_WKV:C_W1], wp_dr[:, C_WKV:C_W1])
        s0_dma = nc.gpsimd.dma_start(s_tmb[:, 0:4], seq_r[:, 0:4])
        nc.gpsimd.dma_start(s_tmb[:, 4:8], seq_r[:, 4:8])
        nc.scalar.dma_start(wp[:, C_W1:C_W2], wp_dr[:, C_W1:C_W2])
        # [w2|w3|wq] held behind the s0 transfer so its DMA-device slot
        # lands after both seq halves (w2 is first needed at H2, ~2us
        # after the forward starts)
        wbig_dma = nc.scalar.dma_start(wp[:, C_W2:WPACK], wp_dr[:, C_W2:WPACK])
        _dep(wbig_dma.ins, s0_dma.ins, sync=False,
             reason="delay bulk weights behind seq halves on the DMA device")

        ident = consts.tile([128, 128], f32, tag="ident")
        make_identity(nc, ident)
        ident_b = consts.tile([128, 128], bf16, tag="ident_b")
        nc.vector.tensor_copy(ident_b[:], ident[:])

        w0_f = wp[:, C_W0 : C_W0 + D]
        w1_f = wp[:, C_W1 : C_W1 + D]
        w2_f = wp[:, C_W2 : C_W2 + D]
        w3_f = wp[:, C_W3 : C_W3 + D]
        wq_f = wp[:, C_WQ : C_WQ + D]
        wk_f = wp[:, C_WKV : C_WKV + D]
        wv_f = wp[:, C_WKV + D : C_WKV + 2 * D]

        # ---- weight prep ----------------------------------------------
        # critical: wk_t -> w0eff = Wk @ w0 (both bf16); Pool evicts keep
        # the DVE free for the S^T eviction
        w0b = big.tile([D, D], bf16, tag="w0b")
        nc.vector.tensor_copy(w0b[:], w0_f)
        pk = pp.tile([128, 128], f32, tag="wtr", bufs=1, name="pk")
        nc.tensor.transpose(pk[:], wk_f, ident)
        wk_tb = big.tile([D, D], bf16, tag="wk_tb")
        nc.scalar.activation(wk_tb[:], pk[:], AF.Copy)
        pk2 = pp.tile([128, 128], f32, tag="wtr", bufs=1, name="pk2")
        nc.tensor.matmul(pk2[:], wk_tb[:], w0b[:])
        w0eff_b = big.tile([D, D], bf16, tag="w0eff_b")
        nc.scalar.activation(w0eff_b[:], pk2[:], AF.Copy)

        # fwd weights (bf16); 2/D folded into layer-3 pair. Only the
        # wpA-resident ones are copied here -- the rest are emitted
        # between fwd stages so their blocked copies can't fill the DVE
        # wait queue ahead of the critical S^T evictions.
        wvsb = big.tile([D, D], bf16, tag="wvsb")
        nc.vector.tensor_scalar_mul(wvsb[:], wv_f, -SC)
        w1b = big.tile([D, D], bf16, tag="w1b")
        w2b = big.tile([D, D], bf16, tag="w2b")
        w3sb = big.tile([D, D], bf16, tag="w3sb")
        wqb = big.tile([D, D], bf16, tag="wqb")

        # ---- S^T (feature-major, bf16) -- mm banks are free until H0;
        # emitted per-tile inside the fwd loop so tile-1's transposes
        # (blocked on the late seq DMA) don't clog the in-order PE queue
        st_b = big.tile([128, N], bf16, tag="st_b")

        def st_transposes(t):
            p = pmm(f"p_st{t}", dt=bf16)
            for j in range(4):
                nc.tensor.matmul(
                    p[:, ts(j, 128)], s_tmb[:, t * 4 + j], ident_b[:],
                    is_transpose=True,
                )
            nc.vector.tensor_copy(st_b[:, ts(t, TT)], p[:])

        # w1^T..w3^T (for the backward chain), one PSUM group; Pool evict
        # (the DVE is loaded with fwd evictions)
        pw = pp.tile([128, 3, 128], f32, tag="wtr", bufs=1, name="pw")
        for i, wsrc in enumerate((w1_f, w2_f, w3_f)):
            nc.tensor.transpose(pw[:, i], wsrc, ident)
        wt_b = big.tile([128, 3, 128], bf16, tag="wt_b")  # w1t,w2t,w3t
        nc.vector.tensor_copy(wt_b[:], pw[:])
        wt1_b, wt2_b, wt3_b = wt_b[:, 0], wt_b[:, 1], wt_b[:, 2]

        # ---- persistent SBUF activations ------------------------------
        a1b = big.tile([128, N], bf16, tag="a1b")
        a2b = big.tile([128, N], bf16, tag="a2b")
        a3b = big.tile([128, N], bf16, tag="a3b")
        h1b = big.tile([128, N], bf16, tag="h1b")
        g3b = big.tile([128, N], bf16, tag="g3b")
        g2b = big.tile([128, N], bf16, tag="g2b")
        g1b = big.tile([128, N], bf16, tag="g1b")
        sp2b = big.tile([128, N], bf16, tag="sp2b")
        sp1b = big.tile([128, N], bf16, tag="sp1b")
        sp0tm = big.tile([128, NCHUNK, 128], bf16, tag="sp0tm")
        g0tm = big.tile([128, NCHUNK, 128], bf16, tag="g0tm")
        a_tm = {
            i: big.tile([128, N], bf16, name=f"atm{i}", tag=f"atm{i}")
            for i in (1, 2, 3)
        }
        g_tm = {
            i: big.tile([128, N], bf16, name=f"gtm{i}", tag=f"gtm{i}")
            for i in (1, 2, 3)
        }

        silu_insts = []
        dsilu_insts = []

        # ---- forward: 2 tiles of 512, Silu -> bf16 directly; stages
        # interleaved across tiles so ACT runs silus back-to-back ------
        h2_hold = {}
        vh = {}
        h1ps = {}
        for t in range(NTI):
            sl = ts(t, TT)
            st_transposes(t)
            vh[t] = pvh(f"vh{t}")
            # V part of the g3 accumulation only needs S^T: start early
            nc.tensor.matmul(vh[t][:], wvsb[:], st_b[:, sl], start=True, stop=False)
            h0 = pmm(f"h0_{t}")
            nc.tensor.matmul(h0[:], w0eff_b[:], st_b[:, sl])
            silu_insts.append(nc.scalar.activation(a1b[:, sl], h0[:], AF.Silu))
            if t == 0:
                nc.vector.tensor_copy(w1b[:], w1_f)
        for t in range(NTI):
            sl = ts(t, TT)
            h1 = pmm(f"h1_{t}")
            nc.tensor.matmul(h1[:], w1b[:], a1b[:, sl])
            silu_insts.append(nc.scalar.activation(a2b[:, sl], h1[:], AF.Silu))
            h1ps[t] = h1
        nc.vector.tensor_copy(w2b[:], w2_f)
        nc.vector.tensor_scalar_mul(w3sb[:], w3_f, SC)
        nc.vector.tensor_copy(wqb[:], wq_f)

        # ---- token-major transposes: full-width, one bank each --------
        def transpose_full(src, dst, name, tag, evict):
            p = pp.tile([128, N], bf16, tag=tag, bufs=2 if tag == "vh" else 1,
                        name=name)
            for c in range(NCHUNK):
                nc.tensor.matmul(
                    p[:, ts(c, 128)], src[:, ts(c, 128)], ident_b[:],
                    is_transpose=True,
                )
            if evict == "dve":
                nc.vector.tensor_copy(dst[:], p[:])
            elif evict == "pool":
                nc.gpsimd.tensor_copy(dst[:], p[:])
            else:
                nc.scalar.activation(dst[:], p[:], AF.Copy)

        # stage 2 with the a1/a2 transposes threaded through so the DVE
        # queue packs [h1b(t0), tr_a1, h1b(t1), x0b, tr_a2, tr_a3] ahead
        # of the chain TTs; all tr evicts on DVE, g3b/P^T/g_tm3 on Pool
        def stage2(t):
            sl = ts(t, TT)
            # keep h1 for the dsilu epoch (no recompute hops later)
            nc.vector.tensor_copy(h1b[:, sl], h1ps[t][:])
            h2 = phold(f"h2_{t}")
            nc.tensor.matmul(h2[:], w2b[:], a2b[:, sl])
            silu_insts.append(nc.scalar.activation(a3b[:, sl], h2[:], AF.Silu))
            h2_hold[t] = h2

        stage2(0)
        transpose_full(a1b, a_tm[1], "tr_a1", "dw", "dve")
        stage2(1)

        x0b = big.tile([128, NT], bf16, tag="x0b")
        px = pmm("p_x0")
        nc.tensor.matmul(px[:], wqb[:], st_b[:, 0:NT])
        nc.vector.tensor_copy(x0b[:], px[:])

        transpose_full(a2b, a_tm[2], "tr_a2", "dw", "dve")

        for t in range(NTI):
            sl = ts(t, TT)
            # g3 = 2/D*(H3 - V) straight out of the bank; Pool evicts
            nc.tensor.matmul(vh[t][:], w3sb[:], a3b[:, sl], start=False, stop=True)
            nc.vector.tensor_copy(g3b[:, sl], vh[t][:])

        ptb = big.tile([128, NT], bf16, tag="ptb")
        px = pmm("p_pt")
        nc.tensor.matmul(px[:], wk_tb[:], x0b[:])
        nc.vector.tensor_copy(ptb[:], px[:])

        # ---- backward: dsilu epoch + chain ----------------------------
        # c2 = w3^T g3 (plain w3^T; g3 already carries 2/D). c2(t1) sits
        # in the idle wtr bank so the t1 chain's c-matmuls don't serialize
        # behind the t0 TT reads in the 2-deep mm rotation
        c2 = {
            0: pmm("c2_0"),
            1: pp.tile([128, 512], f32, tag="wtr", bufs=1, name="c2_1"),
        }
        for t in range(NTI):
            nc.tensor.matmul(c2[t][:], wt3_b, g3b[:, ts(t, TT)])

        # a3 token-major during the load2 window: last DVE evict ahead of
        # the chain TTs
        transpose_full(a3b, a_tm[3], "tr_a3", "vh", "dve")

        # dsilu epoch (table load hidden under c2/transposes); dsilus run
        # back-to-back on ACT: sp2 from held PSUM, sp1 from the h1b SBUF
        # copy, sp0 token-major from recomputed h0_tm (hold banks free
        # right after sp2)
        h0tm = {}
        for t in range(NTI):
            sl = ts(t, TT)
            di = nc.scalar.activation(sp2b[:, sl], h2_hold[t][:], AF.Derivative_silu)
            dsilu_insts.append(di)
            # h0 token-major: chunks via lhsT = st_b into the vacated bank
            h0tm[t] = phold(f"h0tm_{t}")
            for j in range(4):
                c = t * 4 + j
                nc.tensor.matmul(
                    h0tm[t][:, ts(j, 128)], st_b[:, ts(c, 128)], w0eff_b[:]
                )
        for t in range(NTI):
            di = nc.scalar.activation(
                sp1b[:, ts(t, TT)], h1b[:, ts(t, TT)], AF.Derivative_silu
            )
            dsilu_insts.append(di)
        for t in range(NTI):
            di = nc.scalar.activation(
                sp0tm[:, t * 4 : t * 4 + 4],
                h0tm[t][:].rearrange("p (c d) -> p c d", d=128),
                AF.Derivative_silu,
            )
            dsilu_insts.append(di)

        for di in dsilu_insts:
            _dep(di.ins, silu_insts[-1].ins, sync=False, reason="act-table order")

        # chain per tile: g2 -> c1 -> g1 -> c0_tm -> g0_tm
        # all TTs on the DVE: the chain is DVE-serial (6x658) and the
        # dsilu cadence feeds each TT just in time
        c1 = {}
        c0tm = {}
        for t in range(NTI):
            sl = ts(t, TT)
            nc.vector.tensor_mul(g2b[:, sl], c2[t][:], sp2b[:, sl])
            c1[t] = pmm(f"c1_{t}")
            nc.tensor.matmul(c1[t][:], wt2_b, g2b[:, sl])
        for t in range(NTI):
            sl = ts(t, TT)
            nc.vector.tensor_mul(g1b[:, sl], c1[t][:], sp1b[:, sl])
            # c0 token-major: chunk mms with lhsT = g1b chunks
            c0tm[t] = pmm(f"c0tm_{t}")
            for j in range(4):
                c = t * 4 + j
                nc.tensor.matmul(
                    c0tm[t][:, ts(j, 128)], g1b[:, ts(c, 128)], wt1_b
                )
        for t in range(NTI):
            # g0 stays whole on the DVE: the Pool's in-order queue lags
            # and this TT gates M directly
            nc.vector.tensor_mul(
                g0tm[:, t * 4 : t * 4 + 4],
                c0tm[t][:].rearrange("p (c d) -> p c d", d=128),
                sp0tm[:, t * 4 : t * 4 + 4],
            )

        # g3 token-major: Pool evict (the DVE runs the chain TTs by now)
        transpose_full(g3b, g_tm[3], "tr_g3", "dw", "dve")
        # g2/g1 transposes: PE now; the evictions are placed below, after
        # the M path clears the DVE / the dummy reload clears the ACT
        p_g2 = pp.tile([128, N], bf16, tag="vh", bufs=2, name="p_g2")
        for c in range(NCHUNK):
            nc.tensor.matmul(
                p_g2[:, ts(c, 128)], g2b[:, ts(c, 128)], ident_b[:],
                is_transpose=True,
            )
        p_g1 = pp.tile([128, N], bf16, tag="vh", bufs=2, name="p_g1")
        for c in range(NCHUNK):
            nc.tensor.matmul(
                p_g1[:, ts(c, 128)], g1b[:, ts(c, 128)], ident_b[:],
                is_transpose=True,
            )

        # g1 eviction on DVE right after the chain TTs drain (the ACT is
        # saturated by dsilus+dummy); g2 on Pool
        nc.vector.tensor_copy(g_tm[1][:], p_g1[:])

        # reload the silu table during the dW phase, off the tail
        scr3 = consts.tile([128, 1], f32, tag="scr3")
        dummy = nc.scalar.activation(scr3[:], scr[:], AF.Silu)
        _dep(dummy.ins, dsilu_insts[-1].ins, sync=False, reason="act-table order")
        # g2 eviction on ACT in the post-dummy window (GPSIMD cannot
        # touch PSUM on real TRN2)
        g2ev = nc.scalar.activation(g_tm[2][:], p_g2[:], AF.Copy)
        _dep(g2ev.ins, dummy.ins, sync=False, reason="act order")

        # ---- M in its own bank (m_b must not wait on the dW matmuls);
        # dW accumulation bank: slots dW3 | dW2 | dW1 -------------------
        pm = pp.tile([128, 128], f32, tag="wtr", bufs=1, name="pm")
        for c in range(NCHUNK):
            nc.tensor.matmul(
                pm[:], s_tmb[:, c], g0tm[:, c],
                start=(c == 0), stop=(c == NCHUNK - 1),
            )
        m_b = big.tile([D, D], bf16, tag="m_b")
        nc.vector.tensor_copy(m_b[:], pm[:])

        # ---- retrieval: X1 = X0@w0 + P@M, layers 2..4 -----------------
        r1 = big.tile([128, NT], f32r, tag="r1")
        r2 = big.tile([128, NT], f32r, tag="r2")
        r3b = big.tile([128, NT], bf16, tag="r3b")
        o_tm = big.tile([128, NT // 128, 128], f32, tag="o_tm")
        out_r = out_dr.rearrange("(c p) d -> p c d", p=128)

        # dW3/dW2 before px1 (their data is ready mid-chain; px1-term2 is
        # m_b-gated anyway); dW1 after px1 (it waits on the late g1
        # eviction and must not clog the PE queue ahead of X1)
        acc = pp.tile([128, 4, 128], f32, tag="dw", bufs=1, name="dwacc")

        def dw_mms(atm, gtm, slot):
            for c in range(NCHUNK):
                nc.tensor.matmul(
                    acc[:, slot], atm[:, ts(c, 128)], gtm[:, ts(c, 128)],
                    start=(c == 0), stop=(c == NCHUNK - 1),
                )

        dw_mms(a_tm[3], g_tm[3], 0)
        dw_mms(a_tm[2], g_tm[2], 1)

        nh = NT // RH
        px1s = []
        for hh in range(nh):
            sl = ts(hh, RH)
            px = phold(f"px1_{hh}", RH)
            # term 1 (X0 @ w0) has no M dependency
            nc.tensor.matmul(px[:], w0b[:], x0b[:, sl], start=True, stop=False)
            px1s.append(px)
        for hh in range(nh):
            sl = ts(hh, RH)
            px = px1s[hh]
            nc.tensor.matmul(px[:], m_b[:], ptb[:, sl], start=False, stop=True)
            silu_insts.append(nc.scalar.activation(r1[:, sl], px[:], AF.Silu))

        dw_mms(a_tm[1], g_tm[1], 2)
        # u_i = w_i + dW_i (f32r for the retrieval chain; u3 bf16)
        u1r = big.tile([D, D], f32r, tag="u1r")
        u2r = big.tile([D, D], f32r, tag="u2r")
        u3b = big.tile([D, D], bf16, tag="u3b")
        nc.vector.tensor_add(u1r[:], acc[:, 2], w1_f)
        nc.vector.tensor_add(u2r[:], acc[:, 1], w2_f)
        nc.vector.tensor_add(u3b[:], acc[:, 0], w3_f)
        for hh in range(nh):
            sl = ts(hh, RH)
            px = pmm(f"px2_{hh}", RH)
            nc.tensor.matmul(px[:], u1r[:], r1[:, sl])
            silu_insts.append(nc.scalar.activation(r2[:, sl], px[:], AF.Silu))
        for hh in range(nh):
            sl = ts(hh, RH)
            px = pmm(f"px3_{hh}", RH)
            nc.tensor.matmul(px[:], u2r[:], r2[:, sl])
            silu_insts.append(nc.scalar.activation(r3b[:, sl], px[:], AF.Silu))

        # retrieval silus come after the dummy reload
        for si in silu_insts[6:]:
            _dep(si.ins, dummy.ins, sync=False, reason="act-table order")

        # output: po chunks in the long-free hold banks; DVE evicts (the
        # ACT is running retrieval silus until the very end)
        for c in range(NT // 128):
            po = phold(f"po{c}", 128)
            nc.tensor.matmul(po[:], r3b[:, ts(c, 128)], u3b[:])
            nc.vector.tensor_copy(o_tm[:, c], po[:])
            if c % 2 == 1:
                nc.sync.dma_start(out_r[:, c - 1 : c + 1], o_tm[:, c - 1 : c + 1])


_CACHE = {}


def _get_nc():
    if "nc" not in _CACHE:
        _CACHE["nc"] = _build_program()
    return _CACHE["nc"]


def _pack_weights(w0, w1, w2, w3, wq, wkv):
    return np.ascontiguousarray(
        np.concatenate(
            [np.asarray(x, np.float32) for x in (wkv, w0, w1, w2, w3, wq)], axis=1
        )
    )


def kernel(seq, w0, w1, w2, w3, wq, wkv):
    nc = _get_nc()
    seq = np.ascontiguousarray(np.asarray(seq, np.float32))
    wpack = _pack_weights(w0, w1, w2, w3, wq, wkv)

    in_maps = []
    for c in range(NCORES):
        b, h = c // 2, c % 2
        if h == 0:
            s = seq[b]
        else:
            # rotate: retrieval half first; grad sum is order-invariant
            s = np.concatenate([seq[b, NT:], seq[b, :NT]], axis=0)
        in_maps.append({"seq": np.ascontiguousarray(s), "wpack": wpack})

    res = run_bass_kernel_spmd(nc, in_maps, core_ids=list(range(NCORES)))
    _CACHE["last_results"] = res

    out = np.empty((B, N, D), np.float32)
    for c in range(NCORES):
        b, h = c // 2, c % 2
        out[b, h * NT : (h + 1) * NT] = res.results[c]["out"]
    return out


# revision 50
# speedup vs baseline: 1.2197x; 1.0164x over previous
"""Trainium2 Bass kernel for nn_NeuralMemory (B=4, N=1024, D=128, DEPTH=4).

Sharding: 8 cores, core c handles batch b = c//2; the store phase is
computed redundantly by both cores of a pair (grad sum is token-order
invariant; core h=1 sees its batch rotated so its retrieval half is
tokens [0:512) of its view). No cross-core communication.

v2 design vs the f32r baseline:
- bf16 store phase: Silu writes bf16 activations directly (no f32r
  copies, no gpsimd casts); fwd/bwd chain matmuls run bf16.
- 2/D is folded into the H3-layer weights (w3s = 2/D*w3, wv_s =
  -2/D*Wv), so the H3 PSUM accumulation yields g3 = 2/D*(H3-V) directly.
- dsilu inputs are recomputed on the PE (h1' = a1@w1, h0 token-major)
  instead of persisting H0/H1 -> no fwd PSUM evictions for them.
- The last backward step runs token-major: c0_tm = g1b-chunks @ w1^T,
  sp0_tm = dsilu(h0_tm), g0_tm = c0_tm*sp0_tm feeds M = S^T@G0 without
  a transpose on the critical path.
- Inputs arrive via four parallel DMA queues (SWDGE: [wkv|w0] + seq
  half 1 converting f32->bf16; HWDGE SP: seq half 0; HWDGE ACT: w1;
  HWDGE DVE: [w2|w3|wq]).
- Retrieval keeps f32r (u_i = w_i + dW_i in f32r) except the final
  layer (r3/u3 bf16 so the 128-wide output matmuls run at 1 cyc/row).
- ACT-table epochs: all Silus, then all Derivative_silus (table load
  hidden under c2/transposes), then a dummy Silu reload during the dW
  phase so retrieval Silus pay no load.
"""

import numpy as np

import concourse.bass as bass
import concourse.mybir as mybir
import concourse.tile as tile
from concourse import bacc
from concourse.bass import ts
from concourse.bass_utils import run_bass_kernel_spmd
from concourse.masks import make_identity

B, N, D = 4, 1024, 128
NT = 512            # tokens retrieved per core (half a batch)
TT = 512            # store-phase token tile
NTI = N // TT       # store tiles
NCHUNK = N // 128   # token chunks of 128
RH = 256            # retrieval sub-tile
NCORES = 8
SC = 2.0 / D

# wpack column layout (built host-side): [wkv | w0 | w1 | w2 | w3 | wq]
C_WKV, C_W0, C_W1, C_W2, C_W3, C_WQ = 0, 256, 384, 512, 640, 768
WPACK = 896

f32 = mybir.dt.float32
f32r = mybir.dt.float32r
bf16 = mybir.dt.bfloat16

AF = mybir.ActivationFunctionType
ALU = mybir.AluOpType


def _build_program(reps=1):
    nc = bacc.Bacc(
        "TRN2",
        target_bir_lowering=False,
        debug=False,
        enable_asserts=False,
        num_devices=NCORES,
    )

    seq = nc.dram_tensor("seq", [N, D], f32, kind="ExternalInput").ap()
    wp_dr = nc.dram_tensor("wpack", [D, WPACK], f32, kind="ExternalInput").ap()
    out_dr = nc.dram_tensor("out", [NT, D], f32, kind="ExternalOutput").ap()

    with tile.TileContext(nc) as tc:
        for _ in range(reps):
            _emit(tc, seq, wp_dr, out_dr)

    nc.compile()
    return nc


def _emit(tc, seq, wp_dr, out_dr):
    nc = tc.nc
    from contextlib import ExitStack

    from concourse.tile_rust import add_dep_helper as _dep  # type: ignore

    with ExitStack() as ctx:
        consts = ctx.enter_context(tc.tile_pool(name="consts", bufs=1))
        big = ctx.enter_context(tc.tile_pool(name="big", bufs=1))
        # PSUM banks: mm(2) + hold(2) + vh(2) + dw(1) + wtr(1) = 8
        pp = ctx.enter_context(tc.tile_pool(name="pp", bufs=1, space="PSUM"))

        def pmm(name, w=512, dt=f32):
            return pp.tile([128, w], dt, tag="mm", bufs=2, name=name)

        def phold(name, w=512, dt=f32):
            return pp.tile([128, w], dt, tag="hold", bufs=2, name=name)

        def pvh(name, w=512, dt=f32):
            return pp.tile([128, w], dt, tag="vh", bufs=2, name=name)

        def pdw(name, w=512, dt=bf16):
            return pp.tile([128, w], dt, tag="dw", bufs=1, name=name)

        # PE warm-up ASAP: sets pe_busy_start early so real work runs at
        # full clock (>3us ramp)
        wupa = consts.tile([128, 128], f32r, tag="wupa")
        nc.gpsimd.memset(wupa[:].bitcast(f32), 0.0)
        wupp = pp.tile([128, 512], f32, tag="wtr", bufs=1, name="wupp")
        for k in range(3):
            nc.tensor.matmul(
                wupp[:, 0:128], wupa[:], wupa[:],
                skip_group_check=True,
            )

        # tiny scratch silu pulls the first ACT table load off the
        # critical path (runs during the DMAs)
        scr = consts.tile([128, 1], f32, tag="scr")
        scr2 = consts.tile([128, 1], f32, tag="scr2")
        nc.gpsimd.memset(scr[:], 0.0)
        first_silu_load = nc.scalar.activation(scr2[:], scr[:], AF.Silu)

        # ---- input DMAs across queues (emitted before identity setup so
        # the Pool sequencer reaches the SWDGE desc-gens early) ----------
        wp = consts.tile([D, WPACK], f32, tag="wp")
        s_tmb = big.tile([128, NCHUNK, 128], bf16, tag="s_tmb")  # token-major S
        seq_r = seq.rearrange("(c p) d -> p c d", p=128)

        # HWDGE SP: [wkv|w0] f32 (first on the DMA device -> w0eff chain);
        # SWDGE (gpsimd) converts seq f32->bf16, halves in tile order;
        # HWDGE ACT: remaining weights.
        nc.sync.dma_start(wp[:, C_WKV:C_W1], wp_dr[:, C_WKV:C_W1])
        s0_dma = nc.gpsimd.dma_start(s_tmb[:, 0:4], seq_r[:, 0:4])
        nc.gpsimd.dma_start(s_tmb[:, 4:8], seq_r[:, 4:8])
        nc.scalar.dma_start(wp[:, C_W1:C_W2], wp_dr[:, C_W1:C_W2])
        # [w2|w3|wq] held behind the s0 transfer so its DMA-device slot
        # lands after both seq halves (w2 is first needed at H2, ~2us
        # after the forward starts)
        wbig_dma = nc.scalar.dma_start(wp[:, C_W2:WPACK], wp_dr[:, C_W2:WPACK])
        _dep(wbig_dma.ins, s0_dma.ins, sync=False,
             reason="delay bulk weights behind seq halves on the DMA device")

        ident = consts.tile([128, 128], f32, tag="ident")
        make_identity(nc, ident)
        ident_b = consts.tile([128, 128], bf16, tag="ident_b")
        nc.vector.tensor_copy(ident_b[:], ident[:])

        w0_f = wp[:, C_W0 : C_W0 + D]
        w1_f = wp[:, C_W1 : C_W1 + D]
        w2_f = wp[:, C_W2 : C_W2 + D]
        w3_f = wp[:, C_W3 : C_W3 + D]
        wq_f = wp[:, C_WQ : C_WQ + D]
        wk_f = wp[:, C_WKV : C_WKV + D]
        wv_f = wp[:, C_WKV + D : C_WKV + 2 * D]

        # ---- weight prep ----------------------------------------------
        # critical: wk_t -> w0eff = Wk @ w0 (both bf16); Pool evicts keep
        # the DVE free for the S^T eviction
        w0b = big.tile([D, D], bf16, tag="w0b")
        nc.vector.tensor_copy(w0b[:], w0_f)
        pk = pp.tile([128, 128], f32, tag="wtr", bufs=1, name="pk")
        nc.tensor.transpose(pk[:], wk_f, ident)
        wk_tb = big.tile([D, D], bf16, tag="wk_tb")
        nc.scalar.activation(wk_tb[:], pk[:], AF.Copy)
        pk2 = pp.tile([128, 128], f32, tag="wtr", bufs=1, name="pk2")
        nc.tensor.matmul(pk2[:], wk_tb[:], w0b[:])
        w0eff_b = big.tile([D, D], bf16, tag="w0eff_b")
        nc.scalar.activation(w0eff_b[:], pk2[:], AF.Copy)

        # fwd weights (bf16); 2/D folded into layer-3 pair. Only the
        # wpA-resident ones are copied here -- the rest are emitted
        # between fwd stages so their blocked copies can't fill the DVE
        # wait queue ahead of the critical S^T evictions.
        wvsb = big.tile([D, D], bf16, tag="wvsb")
        nc.vector.tensor_scalar_mul(wvsb[:], wv_f, -SC)
        w1b = big.tile([D, D], bf16, tag="w1b")
        w2b = big.tile([D, D], bf16, tag="w2b")
        w3sb = big.tile([D, D], bf16, tag="w3sb")
        wqb = big.tile([D, D], bf16, tag="wqb")

        # ---- S^T (feature-major, bf16) -- mm banks are free until H0;
        # emitted per-tile inside the fwd loop so tile-1's transposes
        # (blocked on the late seq DMA) don't clog the in-order PE queue
        st_b = big.tile([128, N], bf16, tag="st_b")

        def st_transposes(t):
            p = pmm(f"p_st{t}", dt=bf16)
            for j in range(4):
                nc.tensor.matmul(
                    p[:, ts(j, 128)], s_tmb[:, t * 4 + j], ident_b[:],
                    is_transpose=True,
                )
            nc.vector.tensor_copy(st_b[:, ts(t, TT)], p[:])

        # ---- persistent SBUF activations ------------------------------
        a1b = big.tile([128, N], bf16, tag="a1b")
        a2b = big.tile([128, N], bf16, tag="a2b")
        a3b = big.tile([128, N], bf16, tag="a3b")
        h1b = big.tile([128, N], bf16, tag="h1b")
        g3b = big.tile([128, N], bf16, tag="g3b")
        g2b = big.tile([128, N], bf16, tag="g2b")
        g1b = big.tile([128, N], bf16, tag="g1b")
        sp2b = big.tile([128, N], bf16, tag="sp2b")
        sp1b = big.tile([128, N], bf16, tag="sp1b")
        sp0tm = big.tile([128, NCHUNK, 128], bf16, tag="sp0tm")
        g0tm = big.tile([128, NCHUNK, 128], bf16, tag="g0tm")
        a_tm = {
            i: big.tile([128, N], bf16, name=f"atm{i}", tag=f"atm{i}")
            for i in (1, 2, 3)
        }
        g_tm = {
            i: big.tile([128, N], bf16, name=f"gtm{i}", tag=f"gtm{i}")
            for i in (1, 2, 3)
        }

        silu_insts = []
        dsilu_insts = []

        # ---- forward: 2 tiles of 512, Silu -> bf16 directly; stages
        # interleaved across tiles so ACT runs silus back-to-back ------
        h2_hold = {}
        vh = {}
        h1ps = {}
        for t in range(NTI):
            sl = ts(t, TT)
            st_transposes(t)
            vh[t] = pvh(f"vh{t}")
            # V part of the g3 accumulation only needs S^T: start early
            nc.tensor.matmul(vh[t][:], wvsb[:], st_b[:, sl], start=True, stop=False)
            h0 = pmm(f"h0_{t}")
            nc.tensor.matmul(h0[:], w0eff_b[:], st_b[:, sl])
            silu_insts.append(nc.scalar.activation(a1b[:, sl], h0[:], AF.Silu))
            if t == 0:
                nc.vector.tensor_copy(w1b[:], w1_f)
        # w1^T..w3^T for the backward chain; emitted after stage 0 so the
        # DVE runs the st(t1) eviction before this 525ns copy
        pw = pp.tile([128, 3, 128], f32, tag="wtr", bufs=1, name="pw")
        for i, wsrc in enumerate((w1_f, w2_f, w3_f)):
            nc.tensor.transpose(pw[:, i], wsrc, ident)
        wt_b = big.tile([128, 3, 128], bf16, tag="wt_b")  # w1t,w2t,w3t
        nc.vector.tensor_copy(wt_b[:], pw[:])
        wt1_b, wt2_b, wt3_b = wt_b[:, 0], wt_b[:, 1], wt_b[:, 2]
        for t in range(NTI):
            sl = ts(t, TT)
            h1 = pmm(f"h1_{t}")
            nc.tensor.matmul(h1[:], w1b[:], a1b[:, sl])
            silu_insts.append(nc.scalar.activation(a2b[:, sl], h1[:], AF.Silu))
            h1ps[t] = h1
        nc.vector.tensor_copy(w2b[:], w2_f)
        nc.vector.tensor_scalar_mul(w3sb[:], w3_f, SC)
        nc.vector.tensor_copy(wqb[:], wq_f)

        # ---- token-major transposes: full-width, one bank each --------
        def transpose_full(src, dst, name, tag, evict):
            p = pp.tile([128, N], bf16, tag=tag, bufs=2 if tag == "vh" else 1,
                        name=name)
            for c in range(NCHUNK):
                nc.tensor.matmul(
                    p[:, ts(c, 128)], src[:, ts(c, 128)], ident_b[:],
                    is_transpose=True,
                )
            if evict == "dve":
                nc.vector.tensor_copy(dst[:], p[:])
            elif evict == "pool":
                nc.gpsimd.tensor_copy(dst[:], p[:])
            else:
                nc.scalar.activation(dst[:], p[:], AF.Copy)

        # stage 2 with the a1/a2 transposes threaded through so the DVE
        # queue packs [h1b(t0), tr_a1, h1b(t1), x0b, tr_a2, tr_a3] ahead
        # of the chain TTs; all tr evicts on DVE, g3b/P^T/g_tm3 on Pool
        def stage2(t):
            sl = ts(t, TT)
            # keep h1 for the dsilu epoch (no recompute hops later)
            nc.vector.tensor_copy(h1b[:, sl], h1ps[t][:])
            h2 = phold(f"h2_{t}")
            nc.tensor.matmul(h2[:], w2b[:], a2b[:, sl])
            silu_insts.append(nc.scalar.activation(a3b[:, sl], h2[:], AF.Silu))
            h2_hold[t] = h2

        stage2(0)
        transpose_full(a1b, a_tm[1], "tr_a1", "dw", "dve")
        stage2(1)

        x0b = big.tile([128, NT], bf16, tag="x0b")
        px = pmm("p_x0")
        nc.tensor.matmul(px[:], wqb[:], st_b[:, 0:NT])
        nc.vector.tensor_copy(x0b[:], px[:])

        p_a2 = pp.tile([128, N], bf16, tag="dw", bufs=1, name="p_a2")
        for c in range(NCHUNK):
            nc.tensor.matmul(
                p_a2[:, ts(c, 128)], a2b[:, ts(c, 128)], ident_b[:],
                is_transpose=True,
            )

        for t in range(NTI):
            sl = ts(t, TT)
            # g3 = 2/D*(H3 - V) straight out of the bank; Pool evicts
            nc.tensor.matmul(vh[t][:], w3sb[:], a3b[:, sl], start=False, stop=True)
            nc.vector.tensor_copy(g3b[:, sl], vh[t][:])

        ptb = big.tile([128, NT], bf16, tag="ptb")
        px = pmm("p_pt")
        nc.tensor.matmul(px[:], wk_tb[:], x0b[:])
        nc.vector.tensor_copy(ptb[:], px[:])

        # ---- backward: dsilu epoch + chain ----------------------------
        # c2 = w3^T g3 (plain w3^T; g3 already carries 2/D). c2(t1) sits
        # in the idle wtr bank so the t1 chain's c-matmuls don't serialize
        # behind the t0 TT reads in the 2-deep mm rotation
        c2 = {
            0: pmm("c2_0"),
            1: pp.tile([128, 512], f32, tag="wtr", bufs=1, name="c2_1"),
        }
        for t in range(NTI):
            nc.tensor.matmul(c2[t][:], wt3_b, g3b[:, ts(t, TT)])

        # a3 token-major during the load2 window: last DVE evict ahead of
        # the chain TTs; g3's transposes run now, its eviction joins the
        # DVE once the chain TTs drain
        transpose_full(a3b, a_tm[3], "tr_a3", "vh", "dve")
        transpose_full(g3b, g_tm[3], "tr_g3", "vh", "dve")

        # dsilu epoch (table load hidden under c2/transposes); dsilus run
        # back-to-back on ACT: sp2 from held PSUM, sp1 from the h1b SBUF
        # copy, sp0 token-major from recomputed h0_tm (hold banks free
        # right after sp2)
        h0tm = {}
        for t in range(NTI):
            sl = ts(t, TT)
            di = nc.scalar.activation(sp2b[:, sl], h2_hold[t][:], AF.Derivative_silu)
            dsilu_insts.append(di)
            # h0 token-major: chunks via lhsT = st_b into the vacated bank
            h0tm[t] = phold(f"h0tm_{t}")
            for j in range(4):
                c = t * 4 + j
                nc.tensor.matmul(
                    h0tm[t][:, ts(j, 128)], st_b[:, ts(c, 128)], w0eff_b[:]
                )
        for t in range(NTI):
            di = nc.scalar.activation(
                sp1b[:, ts(t, TT)], h1b[:, ts(t, TT)], AF.Derivative_silu
            )
            dsilu_insts.append(di)
        for t in range(NTI):
            di = nc.scalar.activation(
                sp0tm[:, t * 4 : t * 4 + 4],
                h0tm[t][:].rearrange("p (c d) -> p c d", d=128),
                AF.Derivative_silu,
            )
            dsilu_insts.append(di)

        for di in dsilu_insts:
            _dep(di.ins, silu_insts[-1].ins, sync=False, reason="act-table order")

        # chain per tile: g2 -> c1 -> g1 -> c0_tm -> g0_tm
        # all TTs on the DVE: the chain is DVE-serial (6x658) and the
        # dsilu cadence feeds each TT just in time
        c1 = {}
        c0tm = {}
        for t in range(NTI):
            sl = ts(t, TT)
            nc.vector.tensor_mul(g2b[:, sl], c2[t][:], sp2b[:, sl])
            c1[t] = pmm(f"c1_{t}")
            nc.tensor.matmul(c1[t][:], wt2_b, g2b[:, sl])
        for t in range(NTI):
            sl = ts(t, TT)
            nc.vector.tensor_mul(g1b[:, sl], c1[t][:], sp1b[:, sl])
            # c0 token-major: chunk mms with lhsT = g1b chunks
            c0tm[t] = pmm(f"c0tm_{t}")
            for j in range(4):
                c = t * 4 + j
                nc.tensor.matmul(
                    c0tm[t][:, ts(j, 128)], g1b[:, ts(c, 128)], wt1_b
                )
        for t in range(NTI):
            # g0 stays whole on the DVE: the Pool's in-order queue lags
            # and this TT gates M directly
            nc.vector.tensor_mul(
                g0tm[:, t * 4 : t * 4 + 4],
                c0tm[t][:].rearrange("p (c d) -> p c d", d=128),
                sp0tm[:, t * 4 : t * 4 + 4],
            )

        # g2/g1 transposes: PE now; the evictions are placed below, after
        # the M path clears the DVE / the dummy reload clears the ACT
        p_g2 = pp.tile([128, N], bf16, tag="vh", bufs=2, name="p_g2")
        for c in range(NCHUNK):
            nc.tensor.matmul(
                p_g2[:, ts(c, 128)], g2b[:, ts(c, 128)], ident_b[:],
                is_transpose=True,
            )
        p_g1 = pp.tile([128, N], bf16, tag="vh", bufs=2, name="p_g1")
        for c in range(NCHUNK):
            nc.tensor.matmul(
                p_g1[:, ts(c, 128)], g1b[:, ts(c, 128)], ident_b[:],
                is_transpose=True,
            )

        # g1 eviction on DVE right after the chain TTs drain (the ACT is
        # saturated by dsilus+dummy)
        nc.vector.tensor_copy(g_tm[1][:], p_g1[:])

        # reload the silu table during the dW phase, off the tail
        scr3 = consts.tile([128, 1], f32, tag="scr3")
        dummy = nc.scalar.activation(scr3[:], scr[:], AF.Silu)
        _dep(dummy.ins, dsilu_insts[-1].ins, sync=False, reason="act-table order")
        # g2 eviction on ACT in the post-dummy window (GPSIMD cannot
        # touch PSUM on real TRN2)
        g2ev = nc.scalar.activation(g_tm[2][:], p_g2[:], AF.Copy)
        _dep(g2ev.ins, dummy.ins, sync=False, reason="act order")

        # ---- M in its own bank (m_b must not wait on the dW matmuls);
        # dW accumulation bank: slots dW3 | dW2 | dW1 -------------------
        pm = pp.tile([128, 128], f32, tag="wtr", bufs=1, name="pm")
        for c in range(NCHUNK):
            nc.tensor.matmul(
                pm[:], s_tmb[:, c], g0tm[:, c],
                start=(c == 0), stop=(c == NCHUNK - 1),
            )
        m_b = big.tile([D, D], bf16, tag="m_b")
        nc.vector.tensor_copy(m_b[:], pm[:])
        # late a2 transpose eviction once the chain TTs drain
        nc.vector.tensor_copy(a_tm[2][:], p_a2[:])

        # ---- retrieval: X1 = X0@w0 + P@M, layers 2..4 -----------------
        r1 = big.tile([128, NT], f32r, tag="r1")
        r2 = big.tile([128, NT], f32r, tag="r2")
        r3b = big.tile([128, NT], bf16, tag="r3b")
        o_tm = big.tile([128, NT // 128, 128], f32, tag="o_tm")
        out_r = out_dr.rearrange("(c p) d -> p c d", p=128)

        # dW3/dW2 before px1 (their data is ready mid-chain; px1-term2 is
        # m_b-gated anyway); dW1 after px1 (it waits on the late g1
        # eviction and must not clog the PE queue ahead of X1)
        acc = pp.tile([128, 4, 128], f32, tag="wtr", bufs=1, name="dwacc")

        def dw_mms(atm, gtm, slot):
            for c in range(NCHUNK):
                nc.tensor.matmul(
                    acc[:, slot], atm[:, ts(c, 128)], gtm[:, ts(c, 128)],
                    start=(c == 0), stop=(c == NCHUNK - 1),
                )

        dw_mms(a_tm[3], g_tm[3], 0)
        dw_mms(a_tm[2], g_tm[2], 1)

        nh = NT // RH
        px1s = []
        for hh in range(nh):
            sl = ts(hh, RH)
            px = phold(f"px1_{hh}", RH)
            # term 1 (X0 @ w0) has no M dependency
            nc.tensor.matmul(px[:], w0b[:], x0b[:, sl], start=True, stop=False)
            px1s.append(px)
        for hh in range(nh):
            sl = ts(hh, RH)
            px = px1s[hh]
            nc.tensor.matmul(px[:], m_b[:], ptb[:, sl], start=False, stop=True)
            silu_insts.append(nc.scalar.activation(r1[:, sl], px[:], AF.Silu))

        dw_mms(a_tm[1], g_tm[1], 2)
        # u_i = w_i + dW_i (f32r for the retrieval chain; u3 bf16)
        u1r = big.tile([D, D], f32r, tag="u1r")
        u2r = big.tile([D, D], f32r, tag="u2r")
        u3b = big.tile([D, D], bf16, tag="u3b")
        nc.vector.tensor_add(u1r[:], acc[:, 2], w1_f)
        nc.vector.tensor_add(u2r[:], acc[:, 1], w2_f)
        nc.vector.tensor_add(u3b[:], acc[:, 0], w3_f)
        for hh in range(nh):
            sl = ts(hh, RH)
            px = pmm(f"px2_{hh}", RH)
            nc.tensor.matmul(px[:], u1r[:], r1[:, sl])
            silu_insts.append(nc.scalar.activation(r2[:, sl], px[:], AF.Silu))
        for hh in range(nh):
            sl = ts(hh, RH)
            px = pmm(f"px3_{hh}", RH)
            nc.tensor.matmul(px[:], u2r[:], r2[:, sl])
            silu_insts.append(nc.scalar.activation(r3b[:, sl], px[:], AF.Silu))

        # retrieval silus come after the dummy reload
        for si in silu_insts[6:]:
            _dep(si.ins, dummy.ins, sync=False, reason="act-table order")

        # output: po chunks in the long-free hold banks; DVE evicts (the
        # ACT is running retrieval silus until the very end)
        for c in range(NT // 128):
            po = phold(f"po{c}", 128)
            nc.tensor.matmul(po[:], r3b[:, ts(c, 128)], u3b[:])
            nc.vector.tensor_copy(o_tm[:, c], po[:])
            if c % 2 == 1:
                nc.sync.dma_start(out_r[:, c - 1 : c + 1], o_tm[:, c - 1 : c + 1])


_CACHE = {}


def _get_nc():
    if "nc" not in _CACHE:
        _CACHE["nc"] = _build_program()
    return _CACHE["nc"]


def _pack_weights(w0, w1, w2, w3, wq, wkv):
    return np.ascontiguousarray(
        np.concatenate(
            [np.asarray(x, np.float32) for x in (wkv, w0, w1, w2, w3, wq)], axis=1
        )
    )


def kernel(seq, w0, w1, w2, w3, wq, wkv):
    nc = _get_nc()
    seq = np.ascontiguousarray(np.asarray(seq, np.float32))
    wpack = _pack_weights(w0, w1, w2, w3, wq, wkv)

    in_maps = []
    for c in range(NCORES):
        b, h = c // 2, c % 2
        if h == 0:
            s = seq[b]
        else:
            # rotate: retrieval half first; grad sum is order-invariant
            s = np.concatenate([seq[b, NT:], seq[b, :NT]], axis=0)
        in_maps.append({"seq": np.ascontiguousarray(s), "wpack": wpack})

    res = run_bass_kernel_spmd(nc, in_maps, core_ids=list(range(NCORES)))
    _CACHE["last_results"] = res

    out = np.empty((B, N, D), np.float32)
    for c in range(NCORES):
        b, h = c // 2, c % 2
        out[b, h * NT : (h + 1) * NT] = res.results[c]["out"]
    return out


# revision 53
# speedup vs baseline: 1.2458x; 1.0213x over previous
"""Trainium2 Bass kernel for nn_NeuralMemory (B=4, N=1024, D=128, DEPTH=4).

Sharding: 8 cores, core c handles batch b = c//2; the store phase is
computed redundantly by both cores of a pair (grad sum is token-order
invariant; core h=1 sees its batch rotated so its retrieval half is
tokens [0:512) of its view). No cross-core communication.

v2 design vs the f32r baseline:
- bf16 store phase: Silu writes bf16 activations directly (no f32r
  copies, no gpsimd casts); fwd/bwd chain matmuls run bf16.
- 2/D is folded into the H3-layer weights (w3s = 2/D*w3, wv_s =
  -2/D*Wv), so the H3 PSUM accumulation yields g3 = 2/D*(H3-V) directly.
- dsilu inputs are recomputed on the PE (h1' = a1@w1, h0 token-major)
  instead of persisting H0/H1 -> no fwd PSUM evictions for them.
- The last backward step runs token-major: c0_tm = g1b-chunks @ w1^T,
  sp0_tm = dsilu(h0_tm), g0_tm = c0_tm*sp0_tm feeds M = S^T@G0 without
  a transpose on the critical path.
- Inputs arrive via four parallel DMA queues (SWDGE: [wkv|w0] + seq
  half 1 converting f32->bf16; HWDGE SP: seq half 0; HWDGE ACT: w1;
  HWDGE DVE: [w2|w3|wq]).
- Retrieval keeps f32r (u_i = w_i + dW_i in f32r) except the final
  layer (r3/u3 bf16 so the 128-wide output matmuls run at 1 cyc/row).
- ACT-table epochs: all Silus, then all Derivative_silus (table load
  hidden under c2/transposes), then a dummy Silu reload during the dW
  phase so retrieval Silus pay no load.
"""

import numpy as np

import concourse.bass as bass
import concourse.mybir as mybir
import concourse.tile as tile
from concourse import bacc
from concourse.bass import ts
from concourse.bass_utils import run_bass_kernel_spmd
from concourse.masks import make_identity

B, N, D = 4, 1024, 128
NT = 512            # tokens retrieved per core (half a batch)
TT = 512            # store-phase token tile
NTI = N // TT       # store tiles
NCHUNK = N // 128   # token chunks of 128
RH = 256            # retrieval sub-tile
NCORES = 8
SC = 2.0 / D

# wpack column layout (built host-side): [wkv | w0 | w1 | w2 | w3 | wq]
C_WKV, C_W0, C_W1, C_W2, C_W3, C_WQ = 0, 256, 384, 512, 640, 768
WPACK = 896

f32 = mybir.dt.float32
f32r = mybir.dt.float32r
bf16 = mybir.dt.bfloat16

AF = mybir.ActivationFunctionType
ALU = mybir.AluOpType


def _build_program(reps=1):
    nc = bacc.Bacc(
        "TRN2",
        target_bir_lowering=False,
        debug=False,
        enable_asserts=False,
        num_devices=NCORES,
    )

    seq = nc.dram_tensor("seq", [N, D], f32, kind="ExternalInput").ap()
    wp_dr = nc.dram_tensor("wpack", [D, WPACK], f32, kind="ExternalInput").ap()
    out_dr = nc.dram_tensor("out", [NT, D], f32, kind="ExternalOutput").ap()

    with tile.TileContext(nc) as tc:
        for _ in range(reps):
            _emit(tc, seq, wp_dr, out_dr)

    nc.compile()
    return nc


def _emit(tc, seq, wp_dr, out_dr):
    nc = tc.nc
    from contextlib import ExitStack

    from concourse.tile_rust import add_dep_helper as _dep  # type: ignore

    with ExitStack() as ctx:
        consts = ctx.enter_context(tc.tile_pool(name="consts", bufs=1))
        big = ctx.enter_context(tc.tile_pool(name="big", bufs=1))
        # PSUM banks: mm(2) + hold(2) + vh(2) + dw(1) + wtr(1) = 8
        pp = ctx.enter_context(tc.tile_pool(name="pp", bufs=1, space="PSUM"))

        def pmm(name, w=512, dt=f32):
            return pp.tile([128, w], dt, tag="mm", bufs=2, name=name)

        def phold(name, w=512, dt=f32):
            return pp.tile([128, w], dt, tag="hold", bufs=2, name=name)

        def pvh(name, w=512, dt=f32):
            return pp.tile([128, w], dt, tag="vh", bufs=2, name=name)

        def pdw(name, w=512, dt=bf16):
            return pp.tile([128, w], dt, tag="dw", bufs=1, name=name)

        # PE warm-up ASAP: sets pe_busy_start early so real work runs at
        # full clock (>3us ramp)
        wupa = consts.tile([128, 128], f32r, tag="wupa")
        nc.gpsimd.memset(wupa[:].bitcast(f32), 0.0)
        wupp = pp.tile([128, 512], f32, tag="wtr", bufs=1, name="wupp")
        for k in range(3):
            nc.tensor.matmul(
                wupp[:, 0:128], wupa[:], wupa[:],
                skip_group_check=True,
            )

        # tiny scratch silu pulls the first ACT table load off the
        # critical path (runs during the DMAs)
        scr = consts.tile([128, 1], f32, tag="scr")
        scr2 = consts.tile([128, 1], f32, tag="scr2")
        nc.gpsimd.memset(scr[:], 0.0)
        first_silu_load = nc.scalar.activation(scr2[:], scr[:], AF.Silu)

        # ---- input DMAs across queues (emitted before identity setup so
        # the Pool sequencer reaches the SWDGE desc-gens early) ----------
        wp = consts.tile([D, WPACK], f32, tag="wp")
        s_tmb = big.tile([128, NCHUNK, 128], bf16, tag="s_tmb")  # token-major S
        seq_r = seq.rearrange("(c p) d -> p c d", p=128)

        # HWDGE SP: [wkv|w0] f32 (first on the DMA device -> w0eff chain);
        # SWDGE (gpsimd) converts seq f32->bf16, halves in tile order;
        # HWDGE ACT: remaining weights.
        nc.sync.dma_start(wp[:, C_WKV:C_W1], wp_dr[:, C_WKV:C_W1])
        s0_dma = nc.gpsimd.dma_start(s_tmb[:, 0:4], seq_r[:, 0:4])
        nc.gpsimd.dma_start(s_tmb[:, 4:8], seq_r[:, 4:8])
        nc.scalar.dma_start(wp[:, C_W1:C_W2], wp_dr[:, C_W1:C_W2])
        # [w2|w3|wq] held behind the s0 transfer so its DMA-device slot
        # lands after both seq halves (w2 is first needed at H2, ~2us
        # after the forward starts)
        wbig_dma = nc.scalar.dma_start(wp[:, C_W2:WPACK], wp_dr[:, C_W2:WPACK])
        _dep(wbig_dma.ins, s0_dma.ins, sync=False,
             reason="delay bulk weights behind seq halves on the DMA device")

        ident = consts.tile([128, 128], f32, tag="ident")
        make_identity(nc, ident)
        ident_b = consts.tile([128, 128], bf16, tag="ident_b")
        nc.vector.tensor_copy(ident_b[:], ident[:])

        w0_f = wp[:, C_W0 : C_W0 + D]
        w1_f = wp[:, C_W1 : C_W1 + D]
        w2_f = wp[:, C_W2 : C_W2 + D]
        w3_f = wp[:, C_W3 : C_W3 + D]
        wq_f = wp[:, C_WQ : C_WQ + D]
        wk_f = wp[:, C_WKV : C_WKV + D]
        wv_f = wp[:, C_WKV + D : C_WKV + 2 * D]

        # ---- weight prep ----------------------------------------------
        # critical: wk_t -> w0eff = Wk @ w0 (both bf16); Pool evicts keep
        # the DVE free for the S^T eviction
        w0b = big.tile([D, D], bf16, tag="w0b")
        nc.vector.tensor_copy(w0b[:], w0_f)
        pk = pp.tile([128, 128], f32, tag="wtr", bufs=1, name="pk")
        nc.tensor.transpose(pk[:], wk_f, ident)
        wk_tb = big.tile([D, D], bf16, tag="wk_tb")
        nc.scalar.activation(wk_tb[:], pk[:], AF.Copy)
        pk2 = pp.tile([128, 128], f32, tag="wtr", bufs=1, name="pk2")
        nc.tensor.matmul(pk2[:], wk_tb[:], w0b[:])
        w0eff_b = big.tile([D, D], bf16, tag="w0eff_b")
        nc.scalar.activation(w0eff_b[:], pk2[:], AF.Copy)

        # fwd weights (bf16); 2/D folded into layer-3 pair. Only the
        # wpA-resident ones are copied here -- the rest are emitted
        # between fwd stages so their blocked copies can't fill the DVE
        # wait queue ahead of the critical S^T evictions.
        wvsb = big.tile([D, D], bf16, tag="wvsb")
        nc.vector.tensor_scalar_mul(wvsb[:], wv_f, -SC)
        w1b = big.tile([D, D], bf16, tag="w1b")
        w2b = big.tile([D, D], bf16, tag="w2b")
        w3sb = big.tile([D, D], bf16, tag="w3sb")

        # ---- S^T (feature-major, bf16) -- mm banks are free until H0;
        # emitted per-tile inside the fwd loop so tile-1's transposes
        # (blocked on the late seq DMA) don't clog the in-order PE queue
        st_b = big.tile([128, N], bf16, tag="st_b")

        def st_transposes(t):
            p = pmm(f"p_st{t}", dt=bf16)
            for j in range(4):
                nc.tensor.matmul(
                    p[:, ts(j, 128)], s_tmb[:, t * 4 + j], ident_b[:],
                    is_transpose=True,
                )
            nc.vector.tensor_copy(st_b[:, ts(t, TT)], p[:])

        # ---- persistent SBUF activations ------------------------------
        a1b = big.tile([128, N], bf16, tag="a1b")
        a2b = big.tile([128, N], bf16, tag="a2b")
        a3b = big.tile([128, N], bf16, tag="a3b")
        h1b = big.tile([128, N], bf16, tag="h1b")
        g3b = big.tile([128, N], bf16, tag="g3b")
        g2b = big.tile([128, N], bf16, tag="g2b")
        g1b = big.tile([128, N], bf16, tag="g1b")
        sp2b = big.tile([128, N], bf16, tag="sp2b")
        sp1b = big.tile([128, N], bf16, tag="sp1b")
        sp0tm = big.tile([128, NCHUNK, 128], bf16, tag="sp0tm")
        g0tm = big.tile([128, NCHUNK, 128], bf16, tag="g0tm")
        a_tm = {
            i: big.tile([128, N], bf16, name=f"atm{i}", tag=f"atm{i}")
            for i in (1, 2, 3)
        }
        g_tm = {
            i: big.tile([128, N], bf16, name=f"gtm{i}", tag=f"gtm{i}")
            for i in (1, 2, 3)
        }

        silu_insts = []
        dsilu_insts = []

        # ---- forward: 2 tiles of 512, Silu -> bf16 directly; stages
        # interleaved across tiles so ACT runs silus back-to-back ------
        h2_hold = {}
        vh = {}
        h1ps = {}
        for t in range(NTI):
            sl = ts(t, TT)
            st_transposes(t)
            vh[t] = pvh(f"vh{t}")
            # V part of the g3 accumulation only needs S^T: start early
            nc.tensor.matmul(vh[t][:], wvsb[:], st_b[:, sl], start=True, stop=False)
            h0 = pmm(f"h0_{t}")
            nc.tensor.matmul(h0[:], w0eff_b[:], st_b[:, sl])
            silu_insts.append(nc.scalar.activation(a1b[:, sl], h0[:], AF.Silu))
            if t == 0:
                nc.vector.tensor_copy(w1b[:], w1_f)
        # w1^T..w3^T for the backward chain; emitted after stage 0 so the
        # DVE runs the st(t1) eviction before this 525ns copy
        pw = pp.tile([128, 4, 128], f32, tag="wtr", bufs=1, name="pw")
        for i, wsrc in enumerate((w1_f, w2_f, w3_f, wq_f)):
            nc.tensor.transpose(pw[:, i], wsrc, ident)
        wt_b = big.tile([128, 4, 128], bf16, tag="wt_b")  # w1t,w2t,w3t,wqt
        nc.vector.tensor_copy(wt_b[:], pw[:])
        wt1_b, wt2_b, wt3_b = wt_b[:, 0], wt_b[:, 1], wt_b[:, 2]
        wqt_b = wt_b[:, 3]
        # X0 never materializes: WQ0 = wq@w0 and WKQL = wq@Wk^T turn
        # px1-term1 and P^T into direct S^T matmuls (saves the x0
        # eviction and the p_x0 bank). Both land in one wtr tile.
        wq01p = pp.tile([128, 2, 128], f32, tag="wtr", bufs=1, name="wq01p")
        nc.tensor.matmul(wq01p[:, 0], wqt_b, w0b[:])
        nc.tensor.matmul(wq01p[:, 1], wqt_b, wk_tb[:])
        wq01_b = big.tile([128, 2, 128], bf16, tag="wq01_b")
        nc.vector.tensor_copy(wq01_b[:], wq01p[:])
        wq0_b, wkql_b = wq01_b[:, 0], wq01_b[:, 1]
        for t in range(NTI):
            sl = ts(t, TT)
            h1 = pmm(f"h1_{t}")
            nc.tensor.matmul(h1[:], w1b[:], a1b[:, sl])
            silu_insts.append(nc.scalar.activation(a2b[:, sl], h1[:], AF.Silu))
            h1ps[t] = h1
        nc.vector.tensor_copy(w2b[:], w2_f)
        nc.vector.tensor_scalar_mul(w3sb[:], w3_f, SC)

        # ---- token-major transposes: full-width, one bank each --------
        def transpose_full(src, dst, name, tag, evict):
            p = pp.tile([128, N], bf16, tag=tag, bufs=2 if tag == "vh" else 1,
                        name=name)
            for c in range(NCHUNK):
                nc.tensor.matmul(
                    p[:, ts(c, 128)], src[:, ts(c, 128)], ident_b[:],
                    is_transpose=True,
                )
            if evict == "dve":
                nc.vector.tensor_copy(dst[:], p[:])
            elif evict == "pool":
                nc.gpsimd.tensor_copy(dst[:], p[:])
            else:
                nc.scalar.activation(dst[:], p[:], AF.Copy)

        # stage 2 with the a1/a2 transposes threaded through so the DVE
        # queue packs [h1b(t0), tr_a1, h1b(t1), x0b, tr_a2, tr_a3] ahead
        # of the chain TTs; all tr evicts on DVE, g3b/P^T/g_tm3 on Pool
        def stage2(t):
            sl = ts(t, TT)
            # keep h1 for the dsilu epoch (no recompute hops later)
            nc.vector.tensor_copy(h1b[:, sl], h1ps[t][:])
            h2 = phold(f"h2_{t}")
            nc.tensor.matmul(h2[:], w2b[:], a2b[:, sl])
            silu_insts.append(nc.scalar.activation(a3b[:, sl], h2[:], AF.Silu))
            h2_hold[t] = h2

        stage2(0)
        transpose_full(a1b, a_tm[1], "tr_a1", "dw", "dve")
        stage2(1)

        p_a2 = pp.tile([128, N], bf16, tag="dw", bufs=1, name="p_a2")
        for c in range(NCHUNK):
            nc.tensor.matmul(
                p_a2[:, ts(c, 128)], a2b[:, ts(c, 128)], ident_b[:],
                is_transpose=True,
            )

        for t in range(NTI):
            sl = ts(t, TT)
            # g3 = 2/D*(H3 - V) straight out of the bank; Pool evicts
            nc.tensor.matmul(vh[t][:], w3sb[:], a3b[:, sl], start=False, stop=True)
            nc.vector.tensor_copy(g3b[:, sl], vh[t][:])

        # ---- backward: dsilu epoch + chain ----------------------------
        # c2 = w3^T g3 (plain w3^T; g3 already carries 2/D). c2(t1) sits
        # in the idle wtr bank so the t1 chain's c-matmuls don't serialize
        # behind the t0 TT reads in the 2-deep mm rotation
        c2 = {
            0: pmm("c2_0"),
            1: pp.tile([128, 512], f32, tag="wtr", bufs=1, name="c2_1"),
        }
        for t in range(NTI):
            nc.tensor.matmul(c2[t][:], wt3_b, g3b[:, ts(t, TT)])

        # P^T = WKQL^T @ S^T in the wtr bank (after c2_1 in rotation);
        # the eviction lands on ACT in the post-dummy window
        p_pt = pp.tile([128, NT], f32, tag="wtr", bufs=1, name="p_pt")
        nc.tensor.matmul(p_pt[:], wkql_b[:], st_b[:, 0:NT])
        ptb = big.tile([128, NT], bf16, tag="ptb")

        # a3 token-major during the load2 window: last DVE evict ahead of
        # the chain TTs; g3's transposes run now, its eviction joins the
        # DVE once the chain TTs drain
        transpose_full(a3b, a_tm[3], "tr_a3", "vh", "dve")
        transpose_full(g3b, g_tm[3], "tr_g3", "vh", "dve")

        # dsilu epoch (table load hidden under c2/transposes); dsilus run
        # back-to-back on ACT: sp2 from held PSUM, sp1 from the h1b SBUF
        # copy, sp0 token-major from recomputed h0_tm (hold banks free
        # right after sp2)
        h0tm = {}
        for t in range(NTI):
            sl = ts(t, TT)
            di = nc.scalar.activation(sp2b[:, sl], h2_hold[t][:], AF.Derivative_silu)
            dsilu_insts.append(di)
            # h0 token-major: chunks via lhsT = st_b into the vacated bank
            h0tm[t] = phold(f"h0tm_{t}")
            for j in range(4):
                c = t * 4 + j
                nc.tensor.matmul(
                    h0tm[t][:, ts(j, 128)], st_b[:, ts(c, 128)], w0eff_b[:]
                )
        for t in range(NTI):
            di = nc.scalar.activation(
                sp1b[:, ts(t, TT)], h1b[:, ts(t, TT)], AF.Derivative_silu
            )
            dsilu_insts.append(di)
        for t in range(NTI):
            di = nc.scalar.activation(
                sp0tm[:, t * 4 : t * 4 + 4],
                h0tm[t][:].rearrange("p (c d) -> p c d", d=128),
                AF.Derivative_silu,
            )
            dsilu_insts.append(di)

        for di in dsilu_insts:
            _dep(di.ins, silu_insts[-1].ins, sync=False, reason="act-table order")

        # chain per tile: g2 -> c1 -> g1 -> c0_tm -> g0_tm
        # all TTs on the DVE: the chain is DVE-serial (6x658) and the
        # dsilu cadence feeds each TT just in time
        c1 = {}
        c0tm = {}
        for t in range(NTI):
            sl = ts(t, TT)
            nc.vector.tensor_mul(g2b[:, sl], c2[t][:], sp2b[:, sl])
            c1[t] = pmm(f"c1_{t}")
            nc.tensor.matmul(c1[t][:], wt2_b, g2b[:, sl])
        for t in range(NTI):
            sl = ts(t, TT)
            nc.vector.tensor_mul(g1b[:, sl], c1[t][:], sp1b[:, sl])
            # c0 token-major: chunk mms with lhsT = g1b chunks
            c0tm[t] = pmm(f"c0tm_{t}")
            for j in range(4):
                c = t * 4 + j
                nc.tensor.matmul(
                    c0tm[t][:, ts(j, 128)], g1b[:, ts(c, 128)], wt1_b
                )
        for t in range(NTI):
            # g0 stays whole on the DVE: the Pool's in-order queue lags
            # and this TT gates M directly
            nc.vector.tensor_mul(
                g0tm[:, t * 4 : t * 4 + 4],
                c0tm[t][:].rearrange("p (c d) -> p c d", d=128),
                sp0tm[:, t * 4 : t * 4 + 4],
            )

        # g2/g1 transposes: PE now; the evictions are placed below, after
        # the M path clears the DVE / the dummy reload clears the ACT
        p_g2 = pp.tile([128, N], bf16, tag="vh", bufs=2, name="p_g2")
        for c in range(NCHUNK):
            nc.tensor.matmul(
                p_g2[:, ts(c, 128)], g2b[:, ts(c, 128)], ident_b[:],
                is_transpose=True,
            )
        p_g1 = pp.tile([128, N], bf16, tag="vh", bufs=2, name="p_g1")
        for c in range(NCHUNK):
            nc.tensor.matmul(
                p_g1[:, ts(c, 128)], g1b[:, ts(c, 128)], ident_b[:],
                is_transpose=True,
            )

        # g1 eviction on DVE right after the chain TTs drain (the ACT is
        # saturated by dsilus+dummy)
        nc.vector.tensor_copy(g_tm[1][:], p_g1[:])

        # reload the silu table during the dW phase, off the tail
        scr3 = consts.tile([128, 1], f32, tag="scr3")
        dummy = nc.scalar.activation(scr3[:], scr[:], AF.Silu)
        _dep(dummy.ins, dsilu_insts[-1].ins, sync=False, reason="act-table order")
        # g2 eviction on ACT in the post-dummy window (GPSIMD cannot
        # touch PSUM on real TRN2)
        ptb_ev = nc.scalar.activation(ptb[:], p_pt[:], AF.Copy)
        _dep(ptb_ev.ins, dummy.ins, sync=False, reason="act order")
        g2ev = nc.scalar.activation(g_tm[2][:], p_g2[:], AF.Copy)
        _dep(g2ev.ins, dummy.ins, sync=False, reason="act order")

        # ---- M in its own bank (m_b must not wait on the dW matmuls);
        # dW accumulation bank: slots dW3 | dW2 | dW1 -------------------
        pm = pp.tile([128, 128], f32, tag="wtr", bufs=1, name="pm")
        for c in range(NCHUNK):
            nc.tensor.matmul(
                pm[:], s_tmb[:, c], g0tm[:, c],
                start=(c == 0), stop=(c == NCHUNK - 1),
            )
        m_b = big.tile([D, D], bf16, tag="m_b")
        nc.vector.tensor_copy(m_b[:], pm[:])
        # late a2 transpose eviction once the chain TTs drain
        nc.vector.tensor_copy(a_tm[2][:], p_a2[:])

        # ---- retrieval: X1 = X0@w0 + P@M, layers 2..4 -----------------
        r1 = big.tile([128, NT], f32r, tag="r1")
        r2 = big.tile([128, NT], f32r, tag="r2")
        r3b = big.tile([128, NT], bf16, tag="r3b")
        o_tm = big.tile([128, NT // 128, 128], f32, tag="o_tm")
        out_r = out_dr.rearrange("(c p) d -> p c d", p=128)

        # dW3/dW2 before px1 (their data is ready mid-chain; px1-term2 is
        # m_b-gated anyway); dW1 after px1 (it waits on the late g1
        # eviction and must not clog the PE queue ahead of X1)
        acc = pp.tile([128, 4, 128], f32, tag="wtr", bufs=1, name="dwacc")

        def dw_mms(atm, gtm, slot):
            for c in range(NCHUNK):
                nc.tensor.matmul(
                    acc[:, slot], atm[:, ts(c, 128)], gtm[:, ts(c, 128)],
                    start=(c == 0), stop=(c == NCHUNK - 1),
                )

        dw_mms(a_tm[3], g_tm[3], 0)
        dw_mms(a_tm[2], g_tm[2], 1)

        nh = NT // RH
        px1s = []
        for hh in range(nh):
            sl = ts(hh, RH)
            px = phold(f"px1_{hh}", RH)
            # term 1 (X0 @ w0) has no M dependency
            nc.tensor.matmul(px[:], wq0_b, st_b[:, sl], start=True, stop=False)
            px1s.append(px)
        for hh in range(nh):
            sl = ts(hh, RH)
            px = px1s[hh]
            nc.tensor.matmul(px[:], m_b[:], ptb[:, sl], start=False, stop=True)
            silu_insts.append(nc.scalar.activation(r1[:, sl], px[:], AF.Silu))

        dw_mms(a_tm[1], g_tm[1], 2)
        # u_i = w_i + dW_i (f32r for the retrieval chain; u3 bf16)
        u1r = big.tile([D, D], f32r, tag="u1r")
        u2r = big.tile([D, D], f32r, tag="u2r")
        u3b = big.tile([D, D], bf16, tag="u3b")
        nc.vector.tensor_add(u1r[:], acc[:, 2], w1_f)
        nc.vector.tensor_add(u2r[:], acc[:, 1], w2_f)
        nc.vector.tensor_add(u3b[:], acc[:, 0], w3_f)
        for hh in range(nh):
            sl = ts(hh, RH)
            px = pmm(f"px2_{hh}", RH)
            nc.tensor.matmul(px[:], u1r[:], r1[:, sl])
            silu_insts.append(nc.scalar.activation(r2[:, sl], px[:], AF.Silu))
        for hh in range(nh):
            sl = ts(hh, RH)
            px = pmm(f"px3_{hh}", RH)
            nc.tensor.matmul(px[:], u2r[:], r2[:, sl])
            silu_insts.append(nc.scalar.activation(r3b[:, sl], px[:], AF.Silu))

        # retrieval silus come after the dummy reload
        for si in silu_insts[6:]:
            _dep(si.ins, dummy.ins, sync=False, reason="act-table order")

        # output: po chunks in the long-free hold banks; DVE evicts (the
        # ACT is running retrieval silus until the very end)
        for c in range(NT // 128):
            po = phold(f"po{c}", 128)
            nc.tensor.matmul(po[:], r3b[:, ts(c, 128)], u3b[:])
            nc.vector.tensor_copy(o_tm[:, c], po[:])
            if c % 2 == 1:
                nc.sync.dma_start(out_r[:, c - 1 : c + 1], o_tm[:, c - 1 : c + 1])


_CACHE = {}


def _get_nc():
    if "nc" not in _CACHE:
        _CACHE["nc"] = _build_program()
    return _CACHE["nc"]


def _pack_weights(w0, w1, w2, w3, wq, wkv):
    return np.ascontiguousarray(
        np.concatenate(
            [np.asarray(x, np.float32) for x in (wkv, w0, w1, w2, w3, wq)], axis=1
        )
    )


def kernel(seq, w0, w1, w2, w3, wq, wkv):
    nc = _get_nc()
    seq = np.ascontiguousarray(np.asarray(seq, np.float32))
    wpack = _pack_weights(w0, w1, w2, w3, wq, wkv)

    in_maps = []
    for c in range(NCORES):
        b, h = c // 2, c % 2
        if h == 0:
            s = seq[b]
        else:
            # rotate: retrieval half first; grad sum is order-invariant
            s = np.concatenate([seq[b, NT:], seq[b, :NT]], axis=0)
        in_maps.append({"seq": np.ascontiguousarray(s), "wpack": wpack})

    res = run_bass_kernel_spmd(nc, in_maps, core_ids=list(range(NCORES)))
    _CACHE["last_results"] = res

    out = np.empty((B, N, D), np.float32)
    for c in range(NCORES):
        b, h = c // 2, c % 2
        out[b, h * NT : (h + 1) * NT] = res.results[c]["out"]
    return out


# revision 55
# speedup vs baseline: 1.2607x; 1.0120x over previous
"""Trainium2 Bass kernel for nn_NeuralMemory (B=4, N=1024, D=128, DEPTH=4).

Sharding: 8 cores, core c handles batch b = c//2; the store phase is
computed redundantly by both cores of a pair (grad sum is token-order
invariant; core h=1 sees its batch rotated so its retrieval half is
tokens [0:512) of its view). No cross-core communication.

v2 design vs the f32r baseline:
- bf16 store phase: Silu writes bf16 activations directly (no f32r
  copies, no gpsimd casts); fwd/bwd chain matmuls run bf16.
- 2/D is folded into the H3-layer weights (w3s = 2/D*w3, wv_s =
  -2/D*Wv), so the H3 PSUM accumulation yields g3 = 2/D*(H3-V) directly.
- dsilu inputs are recomputed on the PE (h1' = a1@w1, h0 token-major)
  instead of persisting H0/H1 -> no fwd PSUM evictions for them.
- The last backward step runs token-major: c0_tm = g1b-chunks @ w1^T,
  sp0_tm = dsilu(h0_tm), g0_tm = c0_tm*sp0_tm feeds M = S^T@G0 without
  a transpose on the critical path.
- Inputs arrive via four parallel DMA queues (SWDGE: [wkv|w0] + seq
  half 1 converting f32->bf16; HWDGE SP: seq half 0; HWDGE ACT: w1;
  HWDGE DVE: [w2|w3|wq]).
- Retrieval keeps f32r (u_i = w_i + dW_i in f32r) except the final
  layer (r3/u3 bf16 so the 128-wide output matmuls run at 1 cyc/row).
- ACT-table epochs: all Silus, then all Derivative_silus (table load
  hidden under c2/transposes), then a dummy Silu reload during the dW
  phase so retrieval Silus pay no load.
"""

import numpy as np

import concourse.bass as bass
import concourse.mybir as mybir
import concourse.tile as tile
from concourse import bacc
from concourse.bass import ts
from concourse.bass_utils import run_bass_kernel_spmd
from concourse.masks import make_identity

B, N, D = 4, 1024, 128
NT = 512            # tokens retrieved per core (half a batch)
TT = 512            # store-phase token tile
NTI = N // TT       # store tiles
NCHUNK = N // 128   # token chunks of 128
RH = 256            # retrieval sub-tile
NCORES = 8
SC = 2.0 / D

# wpack column layout (built host-side): [wkv | w0 | w1 | w2 | w3 | wq]
C_WKV, C_W0, C_W1, C_W2, C_W3, C_WQ = 0, 256, 384, 512, 640, 768
WPACK = 896

f32 = mybir.dt.float32
f32r = mybir.dt.float32r
bf16 = mybir.dt.bfloat16

AF = mybir.ActivationFunctionType
ALU = mybir.AluOpType


def _build_program(reps=1):
    nc = bacc.Bacc(
        "TRN2",
        target_bir_lowering=False,
        debug=False,
        enable_asserts=False,
        num_devices=NCORES,
    )

    seq = nc.dram_tensor("seq", [N, D], f32, kind="ExternalInput").ap()
    wp_dr = nc.dram_tensor("wpack", [D, WPACK], f32, kind="ExternalInput").ap()
    out_dr = nc.dram_tensor("out", [NT, D], f32, kind="ExternalOutput").ap()

    with tile.TileContext(nc) as tc:
        for _ in range(reps):
            _emit(tc, seq, wp_dr, out_dr)

    nc.compile()
    return nc


def _emit(tc, seq, wp_dr, out_dr):
    nc = tc.nc
    from contextlib import ExitStack

    from concourse.tile_rust import add_dep_helper as _dep  # type: ignore

    with ExitStack() as ctx:
        consts = ctx.enter_context(tc.tile_pool(name="consts", bufs=1))
        big = ctx.enter_context(tc.tile_pool(name="big", bufs=1))
        # PSUM banks: mm(2) + hold(2) + vh(2) + dw(1) + wtr(1) = 8
        pp = ctx.enter_context(tc.tile_pool(name="pp", bufs=1, space="PSUM"))

        def pmm(name, w=512, dt=f32):
            return pp.tile([128, w], dt, tag="mm", bufs=2, name=name)

        def phold(name, w=512, dt=f32):
            return pp.tile([128, w], dt, tag="hold", bufs=2, name=name)

        def pvh(name, w=512, dt=f32):
            return pp.tile([128, w], dt, tag="vh", bufs=2, name=name)

        def pdw(name, w=512, dt=bf16):
            return pp.tile([128, w], dt, tag="dw", bufs=1, name=name)

        # PE warm-up ASAP: sets pe_busy_start early so real work runs at
        # full clock (>3us ramp)
        wupa = consts.tile([128, 128], f32r, tag="wupa")
        nc.gpsimd.memset(wupa[:].bitcast(f32), 0.0)
        wupp = pp.tile([128, 512], f32, tag="wtr", bufs=1, name="wupp")
        for k in range(3):
            nc.tensor.matmul(
                wupp[:, 0:128], wupa[:], wupa[:],
                skip_group_check=True,
            )

        # tiny scratch silu pulls the first ACT table load off the
        # critical path (runs during the DMAs)
        scr = consts.tile([128, 1], f32, tag="scr")
        scr2 = consts.tile([128, 1], f32, tag="scr2")
        nc.gpsimd.memset(scr[:], 0.0)
        first_silu_load = nc.scalar.activation(scr2[:], scr[:], AF.Silu)

        # ---- input DMAs across queues (emitted before identity setup so
        # the Pool sequencer reaches the SWDGE desc-gens early) ----------
        wp = consts.tile([D, WPACK], f32, tag="wp")
        s_tmb = big.tile([128, NCHUNK, 128], bf16, tag="s_tmb")  # token-major S
        seq_r = seq.rearrange("(c p) d -> p c d", p=128)

        # HWDGE SP: [wkv|w0] f32 (first on the DMA device -> w0eff chain);
        # SWDGE (gpsimd) converts seq f32->bf16, halves in tile order;
        # HWDGE ACT: remaining weights.
        nc.sync.dma_start(wp[:, C_WKV:C_W1], wp_dr[:, C_WKV:C_W1])
        s0_dma = nc.gpsimd.dma_start(s_tmb[:, 0:4], seq_r[:, 0:4])
        nc.gpsimd.dma_start(s_tmb[:, 4:8], seq_r[:, 4:8])
        nc.scalar.dma_start(wp[:, C_W1:C_W2], wp_dr[:, C_W1:C_W2])
        # [w2|w3|wq] held behind the s0 transfer so its DMA-device slot
        # lands after both seq halves (w2 is first needed at H2, ~2us
        # after the forward starts)
        wbig_dma = nc.scalar.dma_start(wp[:, C_W2:WPACK], wp_dr[:, C_W2:WPACK])
        _dep(wbig_dma.ins, s0_dma.ins, sync=False,
             reason="delay bulk weights behind seq halves on the DMA device")

        ident = consts.tile([128, 128], f32, tag="ident")
        make_identity(nc, ident)
        ident_b = consts.tile([128, 128], bf16, tag="ident_b")
        nc.vector.tensor_copy(ident_b[:], ident[:])

        w0_f = wp[:, C_W0 : C_W0 + D]
        w1_f = wp[:, C_W1 : C_W1 + D]
        w2_f = wp[:, C_W2 : C_W2 + D]
        w3_f = wp[:, C_W3 : C_W3 + D]
        wq_f = wp[:, C_WQ : C_WQ + D]
        wk_f = wp[:, C_WKV : C_WKV + D]
        wv_f = wp[:, C_WKV + D : C_WKV + 2 * D]

        # ---- weight prep ----------------------------------------------
        # critical: wk_t -> w0eff = Wk @ w0 (both bf16); Pool evicts keep
        # the DVE free for the S^T eviction
        w0b = big.tile([D, D], bf16, tag="w0b")
        nc.vector.tensor_copy(w0b[:], w0_f)
        pk = pp.tile([128, 128], f32, tag="wtr", bufs=1, name="pk")
        nc.tensor.transpose(pk[:], wk_f, ident)
        wk_tb = big.tile([D, D], bf16, tag="wk_tb")
        nc.scalar.activation(wk_tb[:], pk[:], AF.Copy)
        pk2 = pp.tile([128, 128], f32, tag="wtr", bufs=1, name="pk2")
        nc.tensor.matmul(pk2[:], wk_tb[:], w0b[:])
        w0eff_b = big.tile([D, D], bf16, tag="w0eff_b")
        nc.scalar.activation(w0eff_b[:], pk2[:], AF.Copy)

        # fwd weights (bf16); 2/D folded into layer-3 pair. Only the
        # wpA-resident ones are copied here -- the rest are emitted
        # between fwd stages so their blocked copies can't fill the DVE
        # wait queue ahead of the critical S^T evictions.
        wvsb = big.tile([D, D], bf16, tag="wvsb")
        nc.vector.tensor_scalar_mul(wvsb[:], wv_f, -SC)
        w1b = big.tile([D, D], bf16, tag="w1b")
        w2b = big.tile([D, D], bf16, tag="w2b")
        w3sb = big.tile([D, D], bf16, tag="w3sb")

        # ---- S^T (feature-major, bf16) -- mm banks are free until H0;
        # emitted per-tile inside the fwd loop so tile-1's transposes
        # (blocked on the late seq DMA) don't clog the in-order PE queue
        st_b = big.tile([128, N], bf16, tag="st_b")

        def st_transposes(t):
            p = pmm(f"p_st{t}", dt=bf16)
            for j in range(4):
                nc.tensor.matmul(
                    p[:, ts(j, 128)], s_tmb[:, t * 4 + j], ident_b[:],
                    is_transpose=True,
                )
            nc.vector.tensor_copy(st_b[:, ts(t, TT)], p[:])

        # ---- persistent SBUF activations ------------------------------
        a1b = big.tile([128, N], bf16, tag="a1b")
        a2b = big.tile([128, N], bf16, tag="a2b")
        a3b = big.tile([128, N], bf16, tag="a3b")
        h1b = big.tile([128, N], bf16, tag="h1b")
        g3b = big.tile([128, N], bf16, tag="g3b")
        g2b = big.tile([128, N], bf16, tag="g2b")
        g1b = big.tile([128, N], bf16, tag="g1b")
        sp2b = big.tile([128, N], bf16, tag="sp2b")
        sp1b = big.tile([128, N], bf16, tag="sp1b")
        sp0tm = big.tile([128, NCHUNK, 128], bf16, tag="sp0tm")
        g0tm = big.tile([128, NCHUNK, 128], bf16, tag="g0tm")
        a_tm = {
            i: big.tile([128, N], bf16, name=f"atm{i}", tag=f"atm{i}")
            for i in (1, 2, 3)
        }
        g_tm = {
            i: big.tile([128, N], bf16, name=f"gtm{i}", tag=f"gtm{i}")
            for i in (1, 2, 3)
        }

        silu_insts = []
        dsilu_insts = []

        # ---- forward: 2 tiles of 512, Silu -> bf16 directly; stages
        # interleaved across tiles so ACT runs silus back-to-back ------
        h2_hold = {}
        vh = {}
        h1ps = {}
        for t in range(NTI):
            sl = ts(t, TT)
            st_transposes(t)
            vh[t] = pvh(f"vh{t}")
            # V part of the g3 accumulation only needs S^T: start early
            nc.tensor.matmul(vh[t][:], wvsb[:], st_b[:, sl], start=True, stop=False)
            h0 = pmm(f"h0_{t}")
            nc.tensor.matmul(h0[:], w0eff_b[:], st_b[:, sl])
            silu_insts.append(nc.scalar.activation(a1b[:, sl], h0[:], AF.Silu))
            if t == 0:
                nc.vector.tensor_copy(w1b[:], w1_f)
        # w1^T..w3^T for the backward chain; emitted after stage 0 so the
        # DVE runs the st(t1) eviction before this 525ns copy
        pw = pp.tile([128, 4, 128], f32, tag="wtr", bufs=1, name="pw")
        for i, wsrc in enumerate((w1_f, w2_f, w3_f, wq_f)):
            nc.tensor.transpose(pw[:, i], wsrc, ident)
        wt_b = big.tile([128, 4, 128], bf16, tag="wt_b")  # w1t,w2t,w3t,wqt
        nc.vector.tensor_copy(wt_b[:], pw[:])
        wt1_b, wt2_b, wt3_b = wt_b[:, 0], wt_b[:, 1], wt_b[:, 2]
        wqt_b = wt_b[:, 3]
        # X0 never materializes: WQ0 = wq@w0 and WKQL = wq@Wk^T turn
        # px1-term1 and P^T into direct S^T matmuls (saves the x0
        # eviction and the p_x0 bank). Both land in one wtr tile.
        wq01p = pp.tile([128, 2, 128], f32, tag="wtr", bufs=1, name="wq01p")
        nc.tensor.matmul(wq01p[:, 0], wqt_b, w0b[:])
        nc.tensor.matmul(wq01p[:, 1], wqt_b, wk_tb[:])
        wq01_b = big.tile([128, 2, 128], bf16, tag="wq01_b")
        nc.vector.tensor_copy(wq01_b[:], wq01p[:])
        wq0_b, wkql_b = wq01_b[:, 0], wq01_b[:, 1]
        for t in range(NTI):
            sl = ts(t, TT)
            h1 = pmm(f"h1_{t}")
            nc.tensor.matmul(h1[:], w1b[:], a1b[:, sl])
            silu_insts.append(nc.scalar.activation(a2b[:, sl], h1[:], AF.Silu))
            h1ps[t] = h1
        nc.vector.tensor_copy(w2b[:], w2_f)
        nc.vector.tensor_scalar_mul(w3sb[:], w3_f, SC)

        # ---- token-major transposes: full-width, one bank each --------
        def transpose_full(src, dst, name, tag, evict):
            p = pp.tile([128, N], bf16, tag=tag, bufs=2 if tag == "vh" else 1,
                        name=name)
            for c in range(NCHUNK):
                nc.tensor.matmul(
                    p[:, ts(c, 128)], src[:, ts(c, 128)], ident_b[:],
                    is_transpose=True,
                )
            if evict == "dve":
                nc.vector.tensor_copy(dst[:], p[:])
            elif evict == "pool":
                nc.gpsimd.tensor_copy(dst[:], p[:])
            else:
                nc.scalar.activation(dst[:], p[:], AF.Copy)

        # stage 2 with the a1/a2 transposes threaded through so the DVE
        # queue packs [h1b(t0), tr_a1, h1b(t1), x0b, tr_a2, tr_a3] ahead
        # of the chain TTs; all tr evicts on DVE, g3b/P^T/g_tm3 on Pool
        def stage2(t):
            sl = ts(t, TT)
            # keep h1 for the dsilu epoch (no recompute hops later)
            nc.vector.tensor_copy(h1b[:, sl], h1ps[t][:])
            h2 = phold(f"h2_{t}")
            nc.tensor.matmul(h2[:], w2b[:], a2b[:, sl])
            silu_insts.append(nc.scalar.activation(a3b[:, sl], h2[:], AF.Silu))
            h2_hold[t] = h2

        stage2(0)
        transpose_full(a1b, a_tm[1], "tr_a1", "dw", "dve")
        stage2(1)

        p_a2 = pp.tile([128, N], bf16, tag="dw", bufs=1, name="p_a2")
        for c in range(NCHUNK):
            nc.tensor.matmul(
                p_a2[:, ts(c, 128)], a2b[:, ts(c, 128)], ident_b[:],
                is_transpose=True,
            )

        for t in range(NTI):
            sl = ts(t, TT)
            # g3 = 2/D*(H3 - V) straight out of the bank; Pool evicts
            nc.tensor.matmul(vh[t][:], w3sb[:], a3b[:, sl], start=False, stop=True)
            nc.vector.tensor_copy(g3b[:, sl], vh[t][:])

        # ---- backward: dsilu epoch + chain ----------------------------
        # c2 = w3^T g3 (plain w3^T; g3 already carries 2/D). c2(t1) sits
        # in the idle wtr bank so the t1 chain's c-matmuls don't serialize
        # behind the t0 TT reads in the 2-deep mm rotation
        c2 = {
            0: pmm("c2_0"),
            1: pp.tile([128, 512], f32, tag="wtr", bufs=1, name="c2_1"),
        }
        for t in range(NTI):
            nc.tensor.matmul(c2[t][:], wt3_b, g3b[:, ts(t, TT)])

        # P^T = WKQL^T @ S^T in the wtr bank (after c2_1 in rotation);
        # the eviction lands on ACT in the post-dummy window
        p_pt = pp.tile([128, NT], f32, tag="wtr", bufs=1, name="p_pt")
        nc.tensor.matmul(p_pt[:], wkql_b[:], st_b[:, 0:NT])
        ptb = big.tile([128, NT], bf16, tag="ptb")

        # a3 token-major during the load2 window: last DVE evict ahead of
        # the chain TTs; g3's transposes run now, its eviction joins the
        # DVE once the chain TTs drain
        transpose_full(a3b, a_tm[3], "tr_a3", "vh", "dve")

        # dsilu epoch (table load hidden under c2/transposes); dsilus run
        # back-to-back on ACT: sp2 from held PSUM, sp1 from the h1b SBUF
        # copy, sp0 token-major from recomputed h0_tm (hold banks free
        # right after sp2)
        h0tm = {}
        for t in range(NTI):
            sl = ts(t, TT)
            di = nc.scalar.activation(sp2b[:, sl], h2_hold[t][:], AF.Derivative_silu)
            dsilu_insts.append(di)
            # h0 token-major: chunks via lhsT = st_b into the vacated bank
            h0tm[t] = phold(f"h0tm_{t}")
            for j in range(4):
                c = t * 4 + j
                nc.tensor.matmul(
                    h0tm[t][:, ts(j, 128)], st_b[:, ts(c, 128)], w0eff_b[:]
                )
        # g3 token-major transposes into the hold slot vacated by h0tm;
        # its eviction joins the DVE only after the chain TTs drain
        p_tg3 = phold("p_tg3", N, dt=bf16)
        for c in range(NCHUNK):
            nc.tensor.matmul(
                p_tg3[:, ts(c, 128)], g3b[:, ts(c, 128)], ident_b[:],
                is_transpose=True,
            )
        for t in range(NTI):
            di = nc.scalar.activation(
                sp1b[:, ts(t, TT)], h1b[:, ts(t, TT)], AF.Derivative_silu
            )
            dsilu_insts.append(di)
        for t in range(NTI):
            di = nc.scalar.activation(
                sp0tm[:, t * 4 : t * 4 + 4],
                h0tm[t][:].rearrange("p (c d) -> p c d", d=128),
                AF.Derivative_silu,
            )
            dsilu_insts.append(di)

        for di in dsilu_insts:
            _dep(di.ins, silu_insts[-1].ins, sync=False, reason="act-table order")

        # chain per tile: g2 -> c1 -> g1 -> c0_tm -> g0_tm
        # all TTs on the DVE: the chain is DVE-serial (6x658) and the
        # dsilu cadence feeds each TT just in time
        c1 = {}
        c0tm = {}
        for t in range(NTI):
            sl = ts(t, TT)
            nc.vector.tensor_mul(g2b[:, sl], c2[t][:], sp2b[:, sl])
            c1[t] = pmm(f"c1_{t}")
            nc.tensor.matmul(c1[t][:], wt2_b, g2b[:, sl])
        for t in range(NTI):
            sl = ts(t, TT)
            nc.vector.tensor_mul(g1b[:, sl], c1[t][:], sp1b[:, sl])
            # c0 token-major: chunk mms with lhsT = g1b chunks
            c0tm[t] = pmm(f"c0tm_{t}")
            for j in range(4):
                c = t * 4 + j
                nc.tensor.matmul(
                    c0tm[t][:, ts(j, 128)], g1b[:, ts(c, 128)], wt1_b
                )
        for t in range(NTI):
            # g0 stays whole on the DVE: the Pool's in-order queue lags
            # and this TT gates M directly
            nc.vector.tensor_mul(
                g0tm[:, t * 4 : t * 4 + 4],
                c0tm[t][:].rearrange("p (c d) -> p c d", d=128),
                sp0tm[:, t * 4 : t * 4 + 4],
            )

        # g2/g1 transposes: PE now; the evictions are placed below, after
        # the M path clears the DVE / the dummy reload clears the ACT
        p_g2 = pp.tile([128, N], bf16, tag="vh", bufs=2, name="p_g2")
        for c in range(NCHUNK):
            nc.tensor.matmul(
                p_g2[:, ts(c, 128)], g2b[:, ts(c, 128)], ident_b[:],
                is_transpose=True,
            )
        p_g1 = pp.tile([128, N], bf16, tag="vh", bufs=2, name="p_g1")
        for c in range(NCHUNK):
            nc.tensor.matmul(
                p_g1[:, ts(c, 128)], g1b[:, ts(c, 128)], ident_b[:],
                is_transpose=True,
            )

        # g1 eviction on DVE right after the chain TTs drain (the ACT is
        # saturated by dsilus+dummy)
        nc.vector.tensor_copy(g_tm[1][:], p_g1[:])

        # reload the silu table during the dW phase, off the tail
        scr3 = consts.tile([128, 1], f32, tag="scr3")
        dummy = nc.scalar.activation(scr3[:], scr[:], AF.Silu)
        _dep(dummy.ins, dsilu_insts[-1].ins, sync=False, reason="act-table order")
        # g2 eviction on ACT in the post-dummy window (GPSIMD cannot
        # touch PSUM on real TRN2)
        ptb_ev = nc.scalar.activation(ptb[:], p_pt[:], AF.Copy)
        _dep(ptb_ev.ins, dummy.ins, sync=False, reason="act order")
        g2ev = nc.scalar.activation(g_tm[2][:], p_g2[:], AF.Copy)
        _dep(g2ev.ins, dummy.ins, sync=False, reason="act order")

        # ---- M in its own bank (m_b must not wait on the dW matmuls);
        # dW accumulation bank: slots dW3 | dW2 | dW1 -------------------
        pm = pp.tile([128, 128], f32, tag="wtr", bufs=1, name="pm")
        for c in range(NCHUNK):
            nc.tensor.matmul(
                pm[:], s_tmb[:, c], g0tm[:, c],
                start=(c == 0), stop=(c == NCHUNK - 1),
            )
        m_b = big.tile([D, D], bf16, tag="m_b")
        m_b_copy = nc.vector.tensor_copy(m_b[:], pm[:])
        # late transpose evictions once the chain TTs drain; held behind
        # m_b so the scheduler can't starve the X1-critical path
        tg3_ev = nc.vector.tensor_copy(g_tm[3][:], p_tg3[:])
        _dep(tg3_ev.ins, m_b_copy.ins, sync=False, reason="m_b first on DVE")
        nc.vector.tensor_copy(a_tm[2][:], p_a2[:])

        # ---- retrieval: X1 = X0@w0 + P@M, layers 2..4 -----------------
        r1 = big.tile([128, NT], f32r, tag="r1")
        r2 = big.tile([128, NT], f32r, tag="r2")
        r3b = big.tile([128, NT], bf16, tag="r3b")
        o_tm = big.tile([128, NT // 128, 128], f32, tag="o_tm")
        out_r = out_dr.rearrange("(c p) d -> p c d", p=128)

        # dW3/dW2 before px1 (their data is ready mid-chain; px1-term2 is
        # m_b-gated anyway); dW1 after px1 (it waits on the late g1
        # eviction and must not clog the PE queue ahead of X1)
        acc = pp.tile([128, 4, 128], f32, tag="wtr", bufs=1, name="dwacc")

        def dw_mms(atm, gtm, slot):
            for c in range(NCHUNK):
                nc.tensor.matmul(
                    acc[:, slot], atm[:, ts(c, 128)], gtm[:, ts(c, 128)],
                    start=(c == 0), stop=(c == NCHUNK - 1),
                )

        dw_mms(a_tm[1], g_tm[1], 2)

        nh = NT // RH
        px1s = []
        for hh in range(nh):
            sl = ts(hh, RH)
            px = phold(f"px1_{hh}", RH)
            # term 1 (X0 @ w0) has no M dependency
            nc.tensor.matmul(px[:], wq0_b, st_b[:, sl], start=True, stop=False)
            px1s.append(px)
        for hh in range(nh):
            sl = ts(hh, RH)
            px = px1s[hh]
            nc.tensor.matmul(px[:], m_b[:], ptb[:, sl], start=False, stop=True)
            silu_insts.append(nc.scalar.activation(r1[:, sl], px[:], AF.Silu))

        dw_mms(a_tm[3], g_tm[3], 0)
        dw_mms(a_tm[2], g_tm[2], 1)
        # u_i = w_i + dW_i (f32r for the retrieval chain; u3 bf16)
        u1r = big.tile([D, D], f32r, tag="u1r")
        u2r = big.tile([D, D], f32r, tag="u2r")
        u3b = big.tile([D, D], bf16, tag="u3b")
        nc.vector.tensor_add(u1r[:], acc[:, 2], w1_f)
        nc.vector.tensor_add(u3b[:], acc[:, 0], w3_f)
        nc.vector.tensor_add(u2r[:], acc[:, 1], w2_f)
        for hh in range(nh):
            sl = ts(hh, RH)
            px = pmm(f"px2_{hh}", RH)
            nc.tensor.matmul(px[:], u1r[:], r1[:, sl])
            silu_insts.append(nc.scalar.activation(r2[:, sl], px[:], AF.Silu))
        for hh in range(nh):
            sl = ts(hh, RH)
            px = pmm(f"px3_{hh}", RH)
            nc.tensor.matmul(px[:], u2r[:], r2[:, sl])
            silu_insts.append(nc.scalar.activation(r3b[:, sl], px[:], AF.Silu))

        # retrieval silus come after the dummy reload
        for si in silu_insts[6:]:
            _dep(si.ins, dummy.ins, sync=False, reason="act-table order")

        # output: po chunks in the long-free hold banks; DVE evicts (the
        # ACT is running retrieval silus until the very end)
        for c in range(NT // 128):
            po = phold(f"po{c}", 128)
            nc.tensor.matmul(po[:], r3b[:, ts(c, 128)], u3b[:])
            nc.vector.tensor_copy(o_tm[:, c], po[:])
            if c % 2 == 1:
                nc.sync.dma_start(out_r[:, c - 1 : c + 1], o_tm[:, c - 1 : c + 1])


_CACHE = {}


def _get_nc():
    if "nc" not in _CACHE:
        _CACHE["nc"] = _build_program()
    return _CACHE["nc"]


def _pack_weights(w0, w1, w2, w3, wq, wkv):
    return np.ascontiguousarray(
        np.concatenate(
            [np.asarray(x, np.float32) for x in (wkv, w0, w1, w2, w3, wq)], axis=1
        )
    )


def kernel(seq, w0, w1, w2, w3, wq, wkv):
    nc = _get_nc()
    seq = np.ascontiguousarray(np.asarray(seq, np.float32))
    wpack = _pack_weights(w0, w1, w2, w3, wq, wkv)

    in_maps = []
    for c in range(NCORES):
        b, h = c // 2, c % 2
        if h == 0:
            s = seq[b]
        else:
            # rotate: retrieval half first; grad sum is order-invariant
            s = np.concatenate([seq[b, NT:], seq[b, :NT]], axis=0)
        in_maps.append({"seq": np.ascontiguousarray(s), "wpack": wpack})

    res = run_bass_kernel_spmd(nc, in_maps, core_ids=list(range(NCORES)))
    _CACHE["last_results"] = res

    out = np.empty((B, N, D), np.float32)
    for c in range(NCORES):
        b, h = c // 2, c % 2
        out[b, h * NT : (h + 1) * NT] = res.results[c]["out"]
    return out


# revision 63
# speedup vs baseline: 1.2810x; 1.0161x over previous
"""Trainium2 Bass kernel for nn_NeuralMemory (B=4, N=1024, D=128, DEPTH=4).

Sharding: 8 cores, core c handles batch b = c//2; the store phase is
computed redundantly by both cores of a pair (grad sum is token-order
invariant; core h=1 sees its batch rotated so its retrieval half is
tokens [0:512) of its view). No cross-core communication.

v2 design vs the f32r baseline:
- bf16 store phase: Silu writes bf16 activations directly (no f32r
  copies, no gpsimd casts); fwd/bwd chain matmuls run bf16.
- 2/D is folded into the H3-layer weights (w3s = 2/D*w3, wv_s =
  -2/D*Wv), so the H3 PSUM accumulation yields g3 = 2/D*(H3-V) directly.
- dsilu inputs are recomputed on the PE (h1' = a1@w1, h0 token-major)
  instead of persisting H0/H1 -> no fwd PSUM evictions for them.
- The last backward step runs token-major: c0_tm = g1b-chunks @ w1^T,
  sp0_tm = dsilu(h0_tm), g0_tm = c0_tm*sp0_tm feeds M = S^T@G0 without
  a transpose on the critical path.
- Inputs arrive via four parallel DMA queues (SWDGE: [wkv|w0] + seq
  half 1 converting f32->bf16; HWDGE SP: seq half 0; HWDGE ACT: w1;
  HWDGE DVE: [w2|w3|wq]).
- Retrieval keeps f32r (u_i = w_i + dW_i in f32r) except the final
  layer (r3/u3 bf16 so the 128-wide output matmuls run at 1 cyc/row).
- ACT-table epochs: all Silus, then all Derivative_silus (table load
  hidden under c2/transposes), then a dummy Silu reload during the dW
  phase so retrieval Silus pay no load.
"""

import numpy as np

import concourse.bass as bass
import concourse.mybir as mybir
import concourse.tile as tile
from concourse import bacc
from concourse.bass import ts
from concourse.bass_utils import run_bass_kernel_spmd
from concourse.masks import make_identity

B, N, D = 4, 1024, 128
NT = 512            # tokens retrieved per core (half a batch)
TT = 512            # store-phase token tile
NTI = N // TT       # store tiles
NCHUNK = N // 128   # token chunks of 128
RH = 256            # retrieval sub-tile
NCORES = 8
SC = 2.0 / D

# wpack column layout (built host-side): [wkv | w0 | w1 | w2 | w3 | wq]
C_WKV, C_W0, C_W1, C_W2, C_W3, C_WQ = 0, 256, 384, 512, 640, 768
WPACK = 896

f32 = mybir.dt.float32
f32r = mybir.dt.float32r
bf16 = mybir.dt.bfloat16

AF = mybir.ActivationFunctionType
ALU = mybir.AluOpType


def _build_program(reps=1):
    nc = bacc.Bacc(
        "TRN2",
        target_bir_lowering=False,
        debug=False,
        enable_asserts=False,
        num_devices=NCORES,
    )

    seq = nc.dram_tensor("seq", [N, D], f32, kind="ExternalInput").ap()
    wp_dr = nc.dram_tensor("wpack", [D, WPACK], f32, kind="ExternalInput").ap()
    out_dr = nc.dram_tensor("out", [NT, D], f32, kind="ExternalOutput").ap()

    with tile.TileContext(nc) as tc:
        for _ in range(reps):
            _emit(tc, seq, wp_dr, out_dr)

    nc.compile()
    return nc


def _emit(tc, seq, wp_dr, out_dr):
    nc = tc.nc
    from contextlib import ExitStack

    from concourse.tile_rust import add_dep_helper as _dep  # type: ignore

    with ExitStack() as ctx:
        consts = ctx.enter_context(tc.tile_pool(name="consts", bufs=1))
        big = ctx.enter_context(tc.tile_pool(name="big", bufs=1))
        # PSUM banks: mm(2) + hold(2) + vh(2) + dw(1) + wtr(1) = 8
        pp = ctx.enter_context(tc.tile_pool(name="pp", bufs=1, space="PSUM"))

        def pmm(name, w=512, dt=f32):
            return pp.tile([128, w], dt, tag="mm", bufs=2, name=name)

        def phold(name, w=512, dt=f32):
            return pp.tile([128, w], dt, tag="hold", bufs=2, name=name)

        def pvh(name, w=512, dt=f32):
            return pp.tile([128, w], dt, tag="vh", bufs=2, name=name)

        def pdw(name, w=512, dt=bf16):
            return pp.tile([128, w], dt, tag="dw", bufs=1, name=name)

        # PE warm-up ASAP: sets pe_busy_start early so real work runs at
        # full clock (>3us ramp)
        wupa = consts.tile([128, 128], f32r, tag="wupa")
        nc.gpsimd.memset(wupa[:].bitcast(f32), 0.0)
        wupp = pp.tile([128, 512], f32, tag="wtr", bufs=1, name="wupp")
        for k in range(3):
            nc.tensor.matmul(
                wupp[:, 0:128], wupa[:], wupa[:],
                skip_group_check=True,
            )

        # tiny scratch silu pulls the first ACT table load off the
        # critical path (runs during the DMAs)
        scr = consts.tile([128, 1], f32, tag="scr")
        scr2 = consts.tile([128, 1], f32, tag="scr2")
        nc.gpsimd.memset(scr[:], 0.0)
        first_silu_load = nc.scalar.activation(scr2[:], scr[:], AF.Silu)

        # ---- input DMAs across queues (emitted before identity setup so
        # the Pool sequencer reaches the SWDGE desc-gens early) ----------
        wp = consts.tile([D, WPACK], f32, tag="wp")
        s_tmb = big.tile([128, NCHUNK, 128], bf16, tag="s_tmb")  # token-major S
        seq_r = seq.rearrange("(c p) d -> p c d", p=128)

        # HWDGE SP: [wkv|w0] f32 (first on the DMA device -> w0eff chain);
        # SWDGE (gpsimd) converts seq f32->bf16, halves in tile order;
        # HWDGE ACT: remaining weights.
        nc.sync.dma_start(wp[:, C_WKV:C_W1], wp_dr[:, C_WKV:C_W1])
        s0_dma = nc.gpsimd.dma_start(s_tmb[:, 0:4], seq_r[:, 0:4])
        nc.gpsimd.dma_start(s_tmb[:, 4:8], seq_r[:, 4:8])
        nc.scalar.dma_start(wp[:, C_W1:C_W2], wp_dr[:, C_W1:C_W2])
        # [w2|w3|wq] held behind the s0 transfer so its DMA-device slot
        # lands after both seq halves (w2 is first needed at H2, ~2us
        # after the forward starts)
        wbig_dma = nc.scalar.dma_start(wp[:, C_W2:WPACK], wp_dr[:, C_W2:WPACK])
        _dep(wbig_dma.ins, s0_dma.ins, sync=False,
             reason="delay bulk weights behind seq halves on the DMA device")

        ident = consts.tile([128, 128], f32, tag="ident")
        make_identity(nc, ident)
        ident_b = consts.tile([128, 128], bf16, tag="ident_b")
        nc.vector.tensor_copy(ident_b[:], ident[:])

        w0_f = wp[:, C_W0 : C_W0 + D]
        w1_f = wp[:, C_W1 : C_W1 + D]
        w2_f = wp[:, C_W2 : C_W2 + D]
        w3_f = wp[:, C_W3 : C_W3 + D]
        wq_f = wp[:, C_WQ : C_WQ + D]
        wk_f = wp[:, C_WKV : C_WKV + D]
        wv_f = wp[:, C_WKV + D : C_WKV + 2 * D]

        # ---- weight prep ----------------------------------------------
        # critical: wk_t -> w0eff = Wk @ w0 (both bf16); Pool evicts keep
        # the DVE free for the S^T eviction
        w0b = big.tile([D, D], bf16, tag="w0b")
        nc.vector.tensor_copy(w0b[:], w0_f)
        pk = pp.tile([128, 128], f32, tag="wtr", bufs=1, name="pk")
        nc.tensor.transpose(pk[:], wk_f, ident)
        wk_tb = big.tile([D, D], bf16, tag="wk_tb")
        nc.scalar.activation(wk_tb[:], pk[:], AF.Copy)
        pk2 = pp.tile([128, 128], f32, tag="wtr", bufs=1, name="pk2")
        nc.tensor.matmul(pk2[:], wk_tb[:], w0b[:])
        w0eff_b = big.tile([D, D], bf16, tag="w0eff_b")
        nc.scalar.activation(w0eff_b[:], pk2[:], AF.Copy)

        # fwd weights (bf16); 2/D folded into layer-3 pair. Only the
        # wpA-resident ones are copied here -- the rest are emitted
        # between fwd stages so their blocked copies can't fill the DVE
        # wait queue ahead of the critical S^T evictions.
        wvsb = big.tile([D, D], bf16, tag="wvsb")
        nc.vector.tensor_scalar_mul(wvsb[:], wv_f, -SC)
        w1b = big.tile([D, D], bf16, tag="w1b")
        w2b = big.tile([D, D], bf16, tag="w2b")
        w3sb = big.tile([D, D], bf16, tag="w3sb")

        # ---- S^T (feature-major, bf16) -- mm banks are free until H0;
        # emitted per-tile inside the fwd loop so tile-1's transposes
        # (blocked on the late seq DMA) don't clog the in-order PE queue
        st_b = big.tile([128, N], bf16, tag="st_b")

        def st_transposes(t):
            p = pmm(f"p_st{t}", dt=bf16)
            for j in range(4):
                nc.tensor.matmul(
                    p[:, ts(j, 128)], s_tmb[:, t * 4 + j], ident_b[:],
                    is_transpose=True,
                )
            nc.vector.tensor_copy(st_b[:, ts(t, TT)], p[:])

        # ---- persistent SBUF activations ------------------------------
        a1b = big.tile([128, N], bf16, tag="a1b")
        a2b = big.tile([128, N], bf16, tag="a2b")
        a3b = big.tile([128, N], bf16, tag="a3b")
        h1b = big.tile([128, N], bf16, tag="h1b")
        g3b = big.tile([128, N], bf16, tag="g3b")
        g2b = big.tile([128, N], bf16, tag="g2b")
        g1b = big.tile([128, N], bf16, tag="g1b")
        sp2b = big.tile([128, N], bf16, tag="sp2b")
        sp1b = big.tile([128, N], bf16, tag="sp1b")
        sp0tm = big.tile([128, NCHUNK, 128], bf16, tag="sp0tm")
        g0tm = big.tile([128, NCHUNK, 128], bf16, tag="g0tm")
        a_tm = {
            i: big.tile([128, N], bf16, name=f"atm{i}", tag=f"atm{i}")
            for i in (1, 2, 3)
        }
        g_tm = {
            i: big.tile([128, N], bf16, name=f"gtm{i}", tag=f"gtm{i}")
            for i in (1, 2, 3)
        }

        silu_insts = []
        dsilu_insts = []

        # ---- forward: 2 tiles of 512, Silu -> bf16 directly; stages
        # interleaved across tiles so ACT runs silus back-to-back ------
        h2_hold = {}
        vh = {}
        h1ps = {}
        for t in range(NTI):
            sl = ts(t, TT)
            st_transposes(t)
            vh[t] = pvh(f"vh{t}")
            # V part of the g3 accumulation only needs S^T: start early
            nc.tensor.matmul(vh[t][:], wvsb[:], st_b[:, sl], start=True, stop=False)
            h0 = pmm(f"h0_{t}")
            nc.tensor.matmul(h0[:], w0eff_b[:], st_b[:, sl])
            silu_insts.append(nc.scalar.activation(a1b[:, sl], h0[:], AF.Silu))
            if t == 0:
                nc.vector.tensor_copy(w1b[:], w1_f)
        # w1^T..w3^T for the backward chain; emitted after stage 0 so the
        # DVE runs the st(t1) eviction before this 525ns copy
        pw = pp.tile([128, 4, 128], f32, tag="wtr", bufs=1, name="pw")
        for i, wsrc in enumerate((w1_f, w2_f, w3_f, wq_f)):
            nc.tensor.transpose(pw[:, i], wsrc, ident)
        wt_b = big.tile([128, 4, 128], bf16, tag="wt_b")  # w1t,w2t,w3t,wqt
        nc.vector.tensor_copy(wt_b[:], pw[:])
        wt1_b, wt2_b, wt3_b = wt_b[:, 0], wt_b[:, 1], wt_b[:, 2]
        wqt_b = wt_b[:, 3]
        # X0 never materializes: WQ0 = wq@w0 and WKQL = wq@Wk^T turn
        # px1-term1 and P^T into direct S^T matmuls (saves the x0
        # eviction and the p_x0 bank). Both land in one wtr tile.
        wq01p = pp.tile([128, 2, 128], f32, tag="wtr", bufs=1, name="wq01p")
        nc.tensor.matmul(wq01p[:, 0], wqt_b, w0b[:])
        nc.tensor.matmul(wq01p[:, 1], wqt_b, wk_tb[:])
        wq01_b = big.tile([128, 2, 128], bf16, tag="wq01_b")
        nc.vector.tensor_copy(wq01_b[:], wq01p[:])
        wq0_b, wkql_b = wq01_b[:, 0], wq01_b[:, 1]
        for t in range(NTI):
            sl = ts(t, TT)
            h1 = pmm(f"h1_{t}")
            nc.tensor.matmul(h1[:], w1b[:], a1b[:, sl])
            silu_insts.append(nc.scalar.activation(a2b[:, sl], h1[:], AF.Silu))
            h1ps[t] = h1
        nc.vector.tensor_copy(w2b[:], w2_f)
        nc.vector.tensor_scalar_mul(w3sb[:], w3_f, SC)

        # ---- token-major transposes: full-width, one bank each --------
        def transpose_full(src, dst, name, tag, evict):
            p = pp.tile([128, N], bf16, tag=tag, bufs=2 if tag == "vh" else 1,
                        name=name)
            for c in range(NCHUNK):
                nc.tensor.matmul(
                    p[:, ts(c, 128)], src[:, ts(c, 128)], ident_b[:],
                    is_transpose=True,
                )
            if evict == "dve":
                nc.vector.tensor_copy(dst[:], p[:])
            elif evict == "pool":
                nc.gpsimd.tensor_copy(dst[:], p[:])
            else:
                nc.scalar.activation(dst[:], p[:], AF.Copy)

        # stage 2 with the a1/a2 transposes threaded through so the DVE
        # queue packs [h1b(t0), tr_a1, h1b(t1), x0b, tr_a2, tr_a3] ahead
        # of the chain TTs; all tr evicts on DVE, g3b/P^T/g_tm3 on Pool
        def stage2(t):
            sl = ts(t, TT)
            # keep h1 for the dsilu epoch (no recompute hops later)
            nc.vector.tensor_copy(h1b[:, sl], h1ps[t][:])
            h2 = phold(f"h2_{t}")
            nc.tensor.matmul(h2[:], w2b[:], a2b[:, sl])
            silu_insts.append(nc.scalar.activation(a3b[:, sl], h2[:], AF.Silu))
            h2_hold[t] = h2

        stage2(0)
        transpose_full(a1b, a_tm[1], "tr_a1", "dw", "dve")
        stage2(1)

        p_a2 = pp.tile([128, N], bf16, tag="dw", bufs=1, name="p_a2")
        for c in range(NCHUNK):
            nc.tensor.matmul(
                p_a2[:, ts(c, 128)], a2b[:, ts(c, 128)], ident_b[:],
                is_transpose=True,
            )

        for t in range(NTI):
            sl = ts(t, TT)
            # g3 = 2/D*(H3 - V) straight out of the bank; Pool evicts
            nc.tensor.matmul(vh[t][:], w3sb[:], a3b[:, sl], start=False, stop=True)
            nc.vector.tensor_copy(g3b[:, sl], vh[t][:])

        # ---- backward: dsilu epoch + chain ----------------------------
        # c2 = w3^T g3 (plain w3^T; g3 already carries 2/D). c2(t1) sits
        # in the idle wtr bank so the t1 chain's c-matmuls don't serialize
        # behind the t0 TT reads in the 2-deep mm rotation
        c2 = {
            0: pmm("c2_0"),
            1: pp.tile([128, 512], f32, tag="wtr", bufs=1, name="c2_1"),
        }
        for t in range(NTI):
            nc.tensor.matmul(c2[t][:], wt3_b, g3b[:, ts(t, TT)])

        # P^T = WKQL^T @ S^T in the wtr bank (after c2_1 in rotation);
        # the eviction lands on ACT in the post-dummy window
        p_pt = pp.tile([128, NT], f32, tag="wtr", bufs=1, name="p_pt")
        nc.tensor.matmul(p_pt[:], wkql_b[:], st_b[:, 0:NT])
        ptb = big.tile([128, NT], bf16, tag="ptb")

        # a3 token-major during the load2 window: last DVE evict ahead of
        # the chain TTs; g3's transposes run now, its eviction joins the
        # DVE once the chain TTs drain
        transpose_full(a3b, a_tm[3], "tr_a3", "vh", "dve")

        # dsilu epoch (table load hidden under c2/transposes); dsilus run
        # back-to-back on ACT: sp2 from held PSUM, sp1 from the h1b SBUF
        # copy, sp0 token-major from recomputed h0_tm (hold banks free
        # right after sp2)
        h0tm = {}
        for t in range(NTI):
            sl = ts(t, TT)
            di = nc.scalar.activation(sp2b[:, sl], h2_hold[t][:], AF.Derivative_silu)
            dsilu_insts.append(di)
            # h0 token-major: chunks via lhsT = st_b into the vacated bank
            h0tm[t] = phold(f"h0tm_{t}")
            for j in range(4):
                c = t * 4 + j
                nc.tensor.matmul(
                    h0tm[t][:, ts(j, 128)], st_b[:, ts(c, 128)], w0eff_b[:]
                )
        # g3 token-major transposes into the hold slot vacated by h0tm;
        # its eviction joins the DVE only after the chain TTs drain
        p_tg3 = phold("p_tg3", N, dt=bf16)
        for c in range(NCHUNK):
            nc.tensor.matmul(
                p_tg3[:, ts(c, 128)], g3b[:, ts(c, 128)], ident_b[:],
                is_transpose=True,
            )
        for t in range(NTI):
            di = nc.scalar.activation(
                sp1b[:, ts(t, TT)], h1b[:, ts(t, TT)], AF.Derivative_silu
            )
            dsilu_insts.append(di)
        for t in range(NTI):
            di = nc.scalar.activation(
                sp0tm[:, t * 4 : t * 4 + 4],
                h0tm[t][:].rearrange("p (c d) -> p c d", d=128),
                AF.Derivative_silu,
            )
            dsilu_insts.append(di)

        for di in dsilu_insts:
            _dep(di.ins, silu_insts[-1].ins, sync=False, reason="act-table order")

        # chain per tile: g2 -> c1 -> g1 -> c0_tm -> g0_tm
        # all TTs on the DVE: the chain is DVE-serial (6x658) and the
        # dsilu cadence feeds each TT just in time
        c1 = {}
        c0tm = {}
        for t in range(NTI):
            sl = ts(t, TT)
            nc.vector.tensor_mul(g2b[:, sl], c2[t][:], sp2b[:, sl])
            c1[t] = pmm(f"c1_{t}")
            nc.tensor.matmul(c1[t][:], wt2_b, g2b[:, sl])
        for t in range(NTI):
            sl = ts(t, TT)
            nc.vector.tensor_mul(g1b[:, sl], c1[t][:], sp1b[:, sl])
            # c0 token-major: chunk mms with lhsT = g1b chunks
            c0tm[t] = pmm(f"c0tm_{t}")
            for j in range(4):
                c = t * 4 + j
                nc.tensor.matmul(
                    c0tm[t][:, ts(j, 128)], g1b[:, ts(c, 128)], wt1_b
                )
        for t in range(NTI):
            # g0 stays whole on the DVE: the Pool's in-order queue lags
            # and this TT gates M directly
            nc.vector.tensor_mul(
                g0tm[:, t * 4 : t * 4 + 4],
                c0tm[t][:].rearrange("p (c d) -> p c d", d=128),
                sp0tm[:, t * 4 : t * 4 + 4],
            )

        # g2/g1 transposes: PE now; the evictions are placed below, after
        # the M path clears the DVE / the dummy reload clears the ACT
        p_g2 = pp.tile([128, N], bf16, tag="vh", bufs=2, name="p_g2")
        for c in range(NCHUNK):
            nc.tensor.matmul(
                p_g2[:, ts(c, 128)], g2b[:, ts(c, 128)], ident_b[:],
                is_transpose=True,
            )
        p_g1 = pp.tile([128, N], bf16, tag="vh", bufs=2, name="p_g1")
        for c in range(NCHUNK):
            nc.tensor.matmul(
                p_g1[:, ts(c, 128)], g1b[:, ts(c, 128)], ident_b[:],
                is_transpose=True,
            )

        # g1 eviction on DVE right after the chain TTs drain (the ACT is
        # saturated by dsilus+dummy)
        nc.vector.tensor_copy(g_tm[1][:], p_g1[:])

        # reload the silu table during the dW phase, off the tail
        scr3 = consts.tile([128, 1], f32, tag="scr3")
        dummy = nc.scalar.activation(scr3[:], scr[:], AF.Silu)
        _dep(dummy.ins, dsilu_insts[-1].ins, sync=False, reason="act-table order")
        # g2 eviction on ACT in the post-dummy window (GPSIMD cannot
        # touch PSUM on real TRN2)
        ptb_ev = nc.scalar.activation(ptb[:], p_pt[:], AF.Copy)
        _dep(ptb_ev.ins, dummy.ins, sync=False, reason="act order")
        g2ev = nc.scalar.activation(g_tm[2][:], p_g2[:], AF.Copy)
        _dep(g2ev.ins, dummy.ins, sync=False, reason="act order")

        # ---- M in its own bank (m_b must not wait on the dW matmuls);
        # dW accumulation bank: slots dW3 | dW2 | dW1 -------------------
        pm = pp.tile([128, 128], f32, tag="wtr", bufs=1, name="pm")
        for c in range(NCHUNK):
            nc.tensor.matmul(
                pm[:], s_tmb[:, c], g0tm[:, c],
                start=(c == 0), stop=(c == NCHUNK - 1),
            )
        m_b = big.tile([D, D], bf16, tag="m_b")
        m_b_copy = nc.vector.tensor_copy(m_b[:], pm[:])
        # late transpose evictions once the chain TTs drain; held behind
        # m_b so the scheduler can't starve the X1-critical path
        tg3_ev = nc.vector.tensor_copy(g_tm[3][:], p_tg3[:])
        _dep(tg3_ev.ins, m_b_copy.ins, sync=False, reason="m_b first on DVE")
        nc.vector.tensor_copy(a_tm[2][:], p_a2[:])

        # ---- retrieval: X1 = X0@w0 + P@M, layers 2..4 -----------------
        r1 = big.tile([128, NT], f32r, tag="r1")
        r2 = big.tile([128, NT], f32r, tag="r2")
        r3b = big.tile([128, NT], bf16, tag="r3b")
        o_tm = big.tile([128, NT // 128, 128], f32, tag="o_tm")
        out_r = out_dr.rearrange("(c p) d -> p c d", p=128)

        # dW3/dW2 before px1 (their data is ready mid-chain; px1-term2 is
        # m_b-gated anyway); dW1 after px1 (it waits on the late g1
        # eviction and must not clog the PE queue ahead of X1)
        acc = pp.tile([128, 4, 128], f32, tag="wtr", bufs=1, name="dwacc")
        acc1 = pp.tile([128, 128], f32, tag="vh", bufs=2, name="acc1")

        def dw_mms(atm, gtm, slot, bank=None):
            for c in range(NCHUNK):
                nc.tensor.matmul(
                    bank if bank is not None else acc[:, slot],
                    atm[:, ts(c, 128)], gtm[:, ts(c, 128)],
                    start=(c == 0), stop=(c == NCHUNK - 1),
                )

        nh = NT // RH
        px1s = []
        for hh in range(nh):
            sl = ts(hh, RH)
            px = phold(f"px1_{hh}", RH)
            # term 1 (X0 @ w0) has no M dependency
            nc.tensor.matmul(px[:], wq0_b, st_b[:, sl], start=True, stop=False)
            px1s.append(px)
        for hh in range(nh):
            sl = ts(hh, RH)
            px = px1s[hh]
            nc.tensor.matmul(px[:], m_b[:], ptb[:, sl], start=False, stop=True)
            silu_insts.append(nc.scalar.activation(r1[:, sl], px[:], AF.Silu))

        dw_mms(a_tm[1], g_tm[1], 2, bank=acc1[:])
        dw_mms(a_tm[3], g_tm[3], 0)
        dw_mms(a_tm[2], g_tm[2], 1)
        # u_i = w_i + dW_i (f32r for the retrieval chain; u3 bf16)
        u1r = big.tile([D, D], f32r, tag="u1r")
        u2r = big.tile([D, D], f32r, tag="u2r")
        u3b = big.tile([D, D], bf16, tag="u3b")
        nc.vector.tensor_add(u1r[:], acc1[:], w1_f)
        nc.vector.tensor_add(u3b[:], acc[:, 0], w3_f)
        nc.vector.tensor_add(u2r[:], acc[:, 1], w2_f)
        for hh in range(nh):
            sl = ts(hh, RH)
            px = pmm(f"px2_{hh}", RH)
            nc.tensor.matmul(px[:], u1r[:], r1[:, sl])
            silu_insts.append(nc.scalar.activation(r2[:, sl], px[:], AF.Silu))
        for hh in range(nh):
            sl = ts(hh, RH)
            px = pmm(f"px3_{hh}", RH)
            nc.tensor.matmul(px[:], u2r[:], r2[:, sl])
            silu_insts.append(nc.scalar.activation(r3b[:, sl], px[:], AF.Silu))

        # retrieval silus come after the dummy reload
        for si in silu_insts[6:]:
            _dep(si.ins, dummy.ins, sync=False, reason="act-table order")

        # output: po chunks in the long-free hold banks; DVE evicts (the
        # ACT is running retrieval silus until the very end)
        for c in range(NT // 128):
            po = phold(f"po{c}", 128)
            nc.tensor.matmul(po[:], r3b[:, ts(c, 128)], u3b[:])
            nc.vector.tensor_copy(o_tm[:, c], po[:])
            if c % 2 == 1:
                nc.sync.dma_start(out_r[:, c - 1 : c + 1], o_tm[:, c - 1 : c + 1])


_CACHE = {}


def _get_nc():
    if "nc" not in _CACHE:
        _CACHE["nc"] = _build_program()
    return _CACHE["nc"]


def _pack_weights(w0, w1, w2, w3, wq, wkv):
    return np.ascontiguousarray(
        np.concatenate(
            [np.asarray(x, np.float32) for x in (wkv, w0, w1, w2, w3, wq)], axis=1
        )
    )


def kernel(seq, w0, w1, w2, w3, wq, wkv):
    nc = _get_nc()
    seq = np.ascontiguousarray(np.asarray(seq, np.float32))
    wpack = _pack_weights(w0, w1, w2, w3, wq, wkv)

    in_maps = []
    for c in range(NCORES):
        b, h = c // 2, c % 2
        if h == 0:
            s = seq[b]
        else:
            # rotate: retrieval half first; grad sum is order-invariant
            s = np.concatenate([seq[b, NT:], seq[b, :NT]], axis=0)
        in_maps.append({"seq": np.ascontiguousarray(s), "wpack": wpack})

    res = run_bass_kernel_spmd(nc, in_maps, core_ids=list(range(NCORES)))
    _CACHE["last_results"] = res

    out = np.empty((B, N, D), np.float32)
    for c in range(NCORES):
        b, h = c // 2, c % 2
        out[b, h * NT : (h + 1) * NT] = res.results[c]["out"]
    return out
